# revision 1
# baseline (speedup 1.0000x reference)
"""AutoCorrelation (Autoformer) Bass kernel for 8 trn2 NeuronCores.

Problem: B=8, L=4096, H=8, E=64, TOP_K=8.
Sharding: data-parallel over batch (core b handles batch b); the cross-batch
mean for top-k index selection is a [4096]-element AllReduce.

Per-core algorithm (batch slice q,k,v: [L=4096, C=512] fp32, C = H*E):
  1. q,k tile loads interleaved; TensorE transposes -> qT,kT [C, L] bf16
     (PSUM->SBUF copies split across Scalar and Vector engines). The first
     correlation group is woven into the load loop so the PE ramps while DMA
     streams; v is prefetched and cast to bf16 on DVE during the correlation.
  2. Block-Toeplitz correlation on PE (bf16): for block offset m in [0,32):
       D_m[i,j] = sum_{u,c} qT[c,128u+i] * kT[c, 128((u+m)%32)+j]
     accumulated in PSUM tiles [128, 512] (4 block offsets per tile).
  3. mean_value[128m+d] = sum_i [D_m|D_{m+1}][i, i+d]: diagonal sums extracted
     with a DRAM "skewed-pitch" bounce (write pitch 4224, read pitch 4225),
     reads and ones-matmul means software-pipelined into later corr groups.
  4. AllReduce mean_value over the 8 cores in 3 slices; the first two (and
     their slice-top-8) hide under the correlation. Top-8 merge by threshold:
     24 slice candidates, the 16 losers keep weight 0. Per-batch weights
     gathered by a wide iota-compare masked reduce, summed across partitions
     with a ones matmul; exp() folded into the PSUM drain, softmax
     normalization folded into the output copies as a 1/sum scale.
  5. Output = sum_j w_j * roll(v, -d_j) as a 33-block circulant matmul in
     bf16. G [128, 4224] built in SBUF by gpsimd local_scatter of the
     bf16 exp-weights at diagonal offsets computed on DVE.
"""

import os
import sys
import numpy as np

sys.path.insert(0, "/opt/trn_rl_repo")

L = 4096
C = 512  # H*E
B = 8
NT = 32  # L/128 tiles
TOPK = 8
SCALE = 1.0 / 512.0  # mean over H*E
GW = 4224  # 33*128 circulant block columns

_CACHE = {}


def _build():
    import concourse.bass as bass
    import concourse.tile as tile
    from concourse import bacc, mybir

    # Split-AllReduce path (AC_ARSPLIT=1) hides ~5us more under the corr
    # phase but showed a rare intermittent race (1-in-~7 NaN); default to
    # the proven single-AllReduce mid-section.
    AR_SPLIT = os.environ.get("AC_ARSPLIT", "0") == "1"

    f32 = mybir.dt.float32
    bf16 = mybir.dt.bfloat16
    i32 = mybir.dt.int32
    u32 = mybir.dt.uint32
    AF = mybir.ActivationFunctionType
    ALU = mybir.AluOpType

    nc = bacc.Bacc(
        "TRN2", target_bir_lowering=False, debug=False, num_devices=B,
    )

    q_d = nc.dram_tensor("q", [L, C], f32, kind="ExternalInput")
    k_d = nc.dram_tensor("k", [L, C], f32, kind="ExternalInput")
    v_d = nc.dram_tensor("v", [L, C], f32, kind="ExternalInput")
    ident_d = nc.dram_tensor("ident", [128, 128], f32, kind="ExternalInput")
    o_d = nc.dram_tensor("o", [L, C], f32, kind="ExternalOutput")

    # DRAM scratch
    ed_d = nc.dram_tensor("ed", [128 * GW + 128], f32, kind="Internal")
    bri_d = nc.dram_tensor("bri", [32], i32, kind="Internal")
    brf_d = nc.dram_tensor("brf", [32], f32, kind="Internal")
    ccin_d = nc.dram_tensor("ccin", [L], f32, kind="Internal")
    ccout_d = nc.dram_tensor("ccout", [L], f32, kind="Internal",
                             addr_space="Shared")

    with tile.TileContext(nc) as tc:
        with tc.tile_pool(name="const", bufs=1) as constp, \
             tc.tile_pool(name="mvp", bufs=1) as mvp, \
             tc.tile_pool(name="vtp", bufs=1) as vtp:
            ident = constp.tile([128, 128], f32)
            nc.sync.dma_start(ident[:], ident_d[:, :])
            identb = constp.tile([128, 128], bf16)
            nc.vector.tensor_copy(identb[:], ident[:])
            ones = constp.tile([128, 1], f32)
            nc.vector.memset(ones[:], SCALE)
            onesw = constp.tile([128, 1], f32)
            nc.vector.memset(onesw[:], 1.0)
            mv = mvp.tile([1, L], f32)
            VT = vtp.tile([128, NT * C], bf16)  # v tiles, bf16, tile-major

            # W = number of top-k candidates carried through the mid-section.
            # With the split AllReduce, top-8 runs per slice (24 candidates);
            # losers are masked to weight 0 and scatter zeros into G.
            W = 24 if AR_SPLIT else 8

            # data-independent constants for the mid-section (built early,
            # off the critical path):
            # ioWf[p, 32j+f] = 32p + f  (weight-gather compare basis)
            ioW = constp.tile([128, 32 * W], i32)
            nc.gpsimd.iota(ioW[:], [[0, W], [1, 32]], base=0,
                           channel_multiplier=32)
            ioWf = constp.tile([128, 32 * W], f32)
            nc.vector.tensor_copy(ioWf[:], ioW[:])
            # tpW[p, j] = p  (partition index)
            tpW = constp.tile([128, W], i32)
            nc.gpsimd.iota(tpW[:], [[0, W]], base=0, channel_multiplier=1)
            # coff[p, W*c+j] = 1408c (local_scatter chunk offsets)
            coff = constp.tile([128, 3 * W], i32)
            nc.gpsimd.iota(coff[:], [[1408, 3], [0, W]], base=0,
                           channel_multiplier=0)
            # mid-section tiles that are produced during the corr phase
            grand = constp.tile([1, L], f32)
            cval = constp.tile([1, W], f32)
            cidxu = constp.tile([1, W], u32)
            cidx = constp.tile([1, W], i32)

            # -------- Phase 1+2: pipelined load/transpose/correlation -------
            # q,k tile loads interleaved; the first correlation group (mg=0)
            # is woven into the load loop so PE ramps while DMA streams; the
            # shear bounce + mean matmuls are interleaved into the mg loop.
            with tc.tile_pool(name="qkT", bufs=1) as qkTp, \
                 tc.tile_pool(name="ld", bufs=6) as ldp, \
                 tc.tile_pool(name="sh", bufs=4) as shp, \
                 tc.tile_pool(name="trps", bufs=4, space="PSUM") as trpsp, \
                 tc.tile_pool(name="corr", bufs=2, space="PSUM") as corrp, \
                 tc.tile_pool(name="mvps", bufs=2, space="PSUM") as mvpsp, \
                 tc.tile_pool(name="pp", bufs=1) as ppp:
                qT = [qkTp.tile([128, L], bf16, tag=f"qT{g}", name=f"qT{g}")
                      for g in range(4)]
                kT = [qkTp.tile([128, L], bf16, tag=f"kT{g}", name=f"kT{g}")
                      for g in range(4)]
                PP = ppp.tile([128, L], f32)

                def corr_mms(P, mg, u, first):
                    s = 128 * ((u + mg) % NT)
                    last = (u == NT - 1)
                    for g in range(4):
                        lhsT = kT[g][:, 128 * u:128 * (u + 1)]
                        st = first and g == 0
                        lastg = last and g == 3
                        if s <= L - 512:
                            nc.tensor.matmul(
                                P[:, :], lhsT, qT[g][:, s:s + 512],
                                start=st, stop=lastg,
                                skip_group_check=True)
                        else:
                            n1 = L - s
                            nc.tensor.matmul(
                                P[:, 0:n1], lhsT, qT[g][:, s:L],
                                start=st, stop=False,
                                skip_group_check=True)
                            nc.tensor.matmul(
                                P[:, n1:512], lhsT, qT[g][:, 0:512 - n1],
                                start=st, stop=lastg,
                                skip_group_check=True)

                P0 = corrp.tile([128, 512], f32, tag="P", name="P0")
                P1 = corrp.tile([128, 512], f32, tag="P", name="P1")
                for u in range(NT):
                    for (src, dstT, ceng) in ((q_d, qT, nc.scalar),
                                              (k_d, kT, nc.vector)):
                        t_in = ldp.tile([128, C], f32, tag="ld")
                        nc.sync.dma_start(t_in[:], src[128 * u:128 * (u + 1), :])
                        # cast to bf16 on the opposite engine from the copies
                        # so the transposes run at 1 cycle/row instead of 2
                        t_b = ldp.tile([128, C], bf16, tag="ldb")
                        if ceng is nc.scalar:
                            nc.vector.tensor_copy(t_b[:], t_in[:])
                        else:
                            nc.scalar.activation(t_b[:], t_in[:], AF.Identity)
                        for g in range(4):
                            ps = trpsp.tile([128, 128], bf16, tag="tr")
                            nc.tensor.transpose(
                                ps[:], t_b[:, 128 * g:128 * (g + 1)], identb[:])
                            if ceng is nc.scalar:
                                nc.scalar.activation(
                                    dstT[g][:, 128 * u:128 * (u + 1)], ps[:],
                                    AF.Identity)
                            else:
                                nc.vector.tensor_copy(
                                    dstT[g][:, 128 * u:128 * (u + 1)], ps[:])
                    # weave corr groups mg=0 and mg=4 behind the loads: the
                    # window for (mg, u_w) needs q tiles u_w+mg/128..+3 and k
                    # tile u_w, all loaded by iteration u_w+mg/128+3.
                    if u >= 3 and u - 3 <= NT - 5:
                        corr_mms(P0, 0, u - 3, first=(u == 3))
                    if u >= 7 and u - 7 <= NT - 8:
                        corr_mms(P1, 4, u - 7, first=(u == 7))

                # v prefetch + G zero-fill: DMA queued behind q/k, casts on
                # DVE; both complete during the correlation matmuls.
                for u in range(NT):
                    t_in = ldp.tile([128, C], f32, tag="vld")
                    nc.sync.dma_start(t_in[:], v_d[128 * u:128 * (u + 1), :])
                    nc.vector.tensor_copy(VT[:, C * u:C * (u + 1)], t_in[:])

                def drain_chunk(P, ch):
                    nc.scalar.activation(
                        PP[:, 512 * ch:512 * (ch + 1)], P[:], AF.Identity)
                    # shear-write this chunk (pitch GW=4224)
                    nc.sync.dma_start(
                        bass.AP(ed_d, 512 * ch, [[GW, 128], [1, 512]]),
                        PP[:, 512 * ch:512 * (ch + 1)])
                    if ch == 0:
                        # wrap block: ed cols [4096:4224] = PP[:, 0:128]
                        nc.sync.dma_start(
                            bass.AP(ed_d, L, [[GW, 128], [1, 128]]),
                            PP[:, 0:128])

                SHs = {}

                def shear_read(ch):
                    # skewed read: pitch GW+1 extracts diagonals; chunk ch
                    # touches write-chunks ch and ch+1 (skew <= 127), so this
                    # must be issued after drain_chunk(ch+1).
                    SH = shp.tile([128, 512], f32, tag="sh")
                    nc.sync.dma_start(
                        SH[:],
                        bass.AP(ed_d, 512 * ch, [[GW + 1, 128], [1, 512]]))
                    SHs[ch] = SH

                def mean_mm(ch):
                    mps = mvpsp.tile([1, 512], f32, tag="mv")
                    nc.tensor.matmul(
                        mps[:], ones[:], SHs[ch][:], start=True, stop=True)
                    nc.scalar.activation(
                        mv[:, 512 * ch:512 * (ch + 1)], mps[:], AF.Identity)

                # finish mg=0 (wrap rows), then the remaining 7 groups.
                # Shear chunk ch-1 is read back right after drain(ch) lands;
                # its mean-matmul runs mid-way through the NEXT group so it
                # never heads the PE queue while the DMA is in flight. The
                # first AllReduce half launches once mean chunks 0..3 are
                # done and hides under the remaining corr groups.
                def ar_slice(lo, hi):
                    nc.sync.dma_start(ccin_d[lo:hi], mv[:, lo:hi])
                    nc.gpsimd.collective_compute(
                        "AllReduce", mybir.AluOpType.add,
                        replica_groups=[list(range(B))],
                        ins=[ccin_d[lo:hi]], outs=[ccout_d[lo:hi]],
                    )

                def slice_topk(lo, hi, slot):
                    # local top-8 of grand[lo:hi]; indices made global
                    nc.sync.dma_start(grand[:, lo:hi], ccout_d[lo:hi])
                    s = slice(8 * slot, 8 * slot + 8)
                    nc.vector.max_with_indices(cval[:, s], cidxu[:, s],
                                               grand[:, lo:hi])
                    nc.vector.tensor_scalar(cidx[:, s], cidxu[:, s], lo, None,
                                            ALU.add)

                for u in range(NT - 4, NT):
                    corr_mms(P0, 0, u, first=False)
                drain_chunk(P0, 0)
                for u in range(NT - 7, NT):
                    corr_mms(P1, 4, u, first=False)
                drain_chunk(P1, 1)
                shear_read(0)
                for mg in range(8, NT, 4):
                    P = corrp.tile([128, 512], f32, tag="P")
                    ch = mg // 4
                    for u in range(NT):
                        corr_mms(P, mg, u, first=(u == 0))
                        if u == 16 and ch >= 2:
                            mean_mm(ch - 2)
                        if AR_SPLIT and u == 24 and ch == 5:
                            ar_slice(0, 2048)       # hidden under corr
                        if AR_SPLIT and u == 8 and ch == 7:
                            slice_topk(0, 2048, 0)  # hidden under corr
                        if AR_SPLIT and u == 24 and ch == 7:
                            ar_slice(2048, 3072)    # hidden under corr tail
                    drain_chunk(P, ch)
                    shear_read(ch - 1)
                shear_read(7)
                if AR_SPLIT:
                    slice_topk(2048, 3072, 1)
                mean_mm(6)
                mean_mm(7)

            # ------------- Phase 5: final collective slice -------------
            if AR_SPLIT:
                nc.sync.dma_start(ccin_d[3072:L], mv[:, 3072:L])
                nc.gpsimd.collective_compute(
                    "AllReduce", mybir.AluOpType.add,
                    replica_groups=[list(range(B))],
                    ins=[ccin_d[3072:L]], outs=[ccout_d[3072:L]],
                )
            else:
                nc.sync.dma_start(ccin_d[:], mv[:])
                nc.gpsimd.collective_compute(
                    "AllReduce", mybir.AluOpType.add,
                    replica_groups=[list(range(B))],
                    ins=[ccin_d[:]], outs=[ccout_d[:]],
                )
            with tc.tile_pool(name="small", bufs=1) as sp:
                # per-batch mean reshaped [128, 32] for the weight gather
                # (local values only)
                mv2 = sp.tile([128, 32], f32)
                nc.sync.dma_start(mv2[:], bass.AP(ccin_d, 0, [[32, 128], [1, 32]]))

                # ---------------- Phase 6: top-8 ----------------
                if AR_SPLIT:
                    # last slice top-8, then threshold-merge the 24
                    # candidates: winners are the global top-8; losers keep
                    # weight 0 downstream.
                    nc.sync.dma_start(grand[:, 3072:L], ccout_d[3072:L])
                    nc.vector.max_with_indices(cval[:, 16:24], cidxu[:, 16:24],
                                               grand[:, 3072:L])
                    nc.vector.tensor_scalar(cidx[:, 16:24], cidxu[:, 16:24],
                                            3072, None, ALU.add)
                    t8v = sp.tile([1, 8], f32)
                    t8i = sp.tile([1, 8], u32)
                    nc.vector.max_with_indices(t8v[:], t8i[:], cval[:])
                    tau = sp.tile([1, 1], f32)
                    nc.vector.tensor_reduce(tau[:], t8v[:],
                                            mybir.AxisListType.X, ALU.min)
                    mask = sp.tile([1, W], f32)
                    nc.vector.tensor_tensor(
                        mask[:], cval[:],
                        bass.AP(tau.tensor, 0, [[1, 1], [0, W]]), ALU.is_ge)
                else:
                    nc.sync.dma_start(grand[:], ccout_d[:])
                    nc.vector.max_with_indices(cval[:], cidxu[:], grand[:])
                    nc.vector.tensor_copy(cidx[:], cidxu[:])

                # ------------- Phase 7: weights + softmax -------------
                BOUNCE_BC = os.environ.get("AC_BCB", "0") == "1"
                idxb = sp.tile([128, W], i32)
                if BOUNCE_BC:
                    # partition broadcast via DRAM bounce (stride-0 read)
                    nc.sync.dma_start(bri_d[0:W], cidx[:])
                    nc.sync.dma_start(idxb[:],
                                      bass.AP(bri_d, 0, [[0, 128], [1, W]]))
                else:
                    nc.gpsimd.partition_broadcast(idxb[:], cidx[:],
                                                  channels=128)
                idxf = sp.tile([128, W], f32)
                nc.vector.tensor_copy(idxf[:], idxb[:])

                # wpW[p, j] = sum_f [32p+f == d_j] * mv2[p, f] in one wide
                # masked-reduce over [128, W, 32] broadcast access patterns.
                def ap3(t, d):
                    return bass.AP(t.tensor, 0, [[t.shape[1], 128]] + d)
                cmpm = sp.tile([128, 32 * W], f32)
                nc.vector.tensor_tensor(
                    ap3(cmpm, [[32, W], [1, 32]]),
                    ap3(ioWf, [[32, W], [1, 32]]),
                    ap3(idxf, [[1, W], [0, 32]]), ALU.is_equal)
                prods = sp.tile([128, 32 * W], f32)
                nc.vector.tensor_tensor(
                    ap3(prods, [[32, W], [1, 32]]),
                    ap3(cmpm, [[32, W], [1, 32]]),
                    ap3(mv2, [[0, W], [1, 32]]), ALU.mult)
                wpW = sp.tile([128, W], f32)
                nc.vector.tensor_reduce(
                    wpW[:], ap3(prods, [[32, W], [1, 32]]),
                    mybir.AxisListType.X, ALU.add)
                # cross-partition sum via ones matmul; Exp folded into the
                # PSUM drain; non-top-8 candidates masked to 0. Softmax
                # normalization is folded into the output copies (scale by
                # 1/sum), so G holds unnormalized exp(w).
                with tc.tile_pool(name="wps", bufs=1, space="PSUM") as wpsp:
                    wps = wpsp.tile([1, W], f32)
                    nc.tensor.matmul(wps[:], onesw[:], wpW[:],
                                     start=True, stop=True)
                    wexp = sp.tile([1, W], f32)
                    nc.scalar.activation(wexp[:], wps[:], AF.Exp)
                if AR_SPLIT:
                    nc.vector.tensor_tensor(wexp[:], wexp[:], mask[:],
                                            ALU.mult)
                wsum = sp.tile([1, 1], f32)
                nc.vector.tensor_reduce(wsum[:], wexp[:], mybir.AxisListType.X,
                                        ALU.add)
                wrec = sp.tile([1, 1], f32)
                nc.vector.reciprocal(wrec[:], wsum[:])
                wrecb = sp.tile([128, 1], f32)
                if BOUNCE_BC:
                    nc.sync.dma_start(brf_d[0:W], wexp[:])
                    nc.sync.dma_start(brf_d[W:W + 1], wrec[:])
                    nc.sync.dma_start(wrecb[:],
                                      bass.AP(brf_d, W, [[0, 128], [1, 1]]))
                else:
                    nc.gpsimd.partition_broadcast(wrecb[:], wrec[:],
                                                  channels=128)

                # -------- Phase 8: G diag offsets + local_scatter --------
                # offj[p,j] = (d - (d & 127)) + ((p - (d & 127)) & 255)
                rj = sp.tile([128, W], i32)
                nc.vector.tensor_scalar(rj[:], idxb[:], 127, None,
                                        ALU.bitwise_and)
                dmr = sp.tile([128, W], i32)
                nc.vector.tensor_tensor(dmr[:], idxb[:], rj[:], ALU.subtract)
                tmr = sp.tile([128, W], i32)
                nc.vector.tensor_tensor(tmr[:], tpW[:], rj[:], ALU.subtract)
                nc.vector.tensor_scalar(tmr[:], tmr[:], 255, None,
                                        ALU.bitwise_and)
                offj = sp.tile([128, W], i32)
                nc.vector.tensor_tensor(offj[:], dmr[:], tmr[:], ALU.add)

                wvals = sp.tile([128, W], f32)
                if BOUNCE_BC:
                    nc.sync.dma_start(wvals[:],
                                      bass.AP(brf_d, 0, [[0, 128], [1, W]]))
                else:
                    nc.gpsimd.partition_broadcast(wvals[:], wexp[:],
                                                  channels=128)
                wb = sp.tile([128, W], bf16)
                nc.vector.tensor_copy(wb[:], wvals[:])

                # local_scatter builds G directly in SBUF (zeroes dst, then
                # dst[p, idx[p,j]] = wb[p,j]); 3 chunks of 1408 columns to fit
                # the GPSIMD local-RAM limit; out-of-chunk indices go negative
                # (ignored); masked candidates scatter zeros (harmless).
                # Index prep for all 3 chunks fused in [128, 3W].
                G = sp.tile([128, GW], bf16)
                i16 = mybir.dt.int16
                tall = sp.tile([128, 3 * W], i32)
                nc.vector.tensor_tensor(
                    ap3(tall, [[W, 3], [1, W]]),
                    ap3(offj, [[0, 3], [1, W]]),
                    ap3(coff, [[W, 3], [1, W]]), ALU.subtract)
                gall = sp.tile([128, 3 * W], i32)
                nc.vector.tensor_scalar(gall[:], tall[:], 1408, None,
                                        ALU.is_ge)
                nc.vector.tensor_scalar(gall[:], gall[:], 8192, None,
                                        ALU.mult)
                nc.vector.tensor_tensor(tall[:], tall[:], gall[:],
                                        ALU.subtract)
                idx16 = sp.tile([128, 3 * W], i16)
                nc.vector.tensor_copy(idx16[:], tall[:])
                for c in range(3):
                    nc.gpsimd.local_scatter(
                        G[:, 1408 * c:1408 * (c + 1)], wb[:],
                        idx16[:, W * c:W * (c + 1)],
                        channels=128, num_elems=1408, num_idxs=W)

                if os.environ.get("AC_DBG", "0") == "1" and AR_SPLIT:
                    cidxf = sp.tile([1, W], f32)
                    nc.vector.tensor_copy(cidxf[:], cidx[:])
                    nc.sync.dma_start(bass.AP(o_d, 0, [[W, 1], [1, W]]), cval[:])
                    nc.sync.dma_start(bass.AP(o_d, 32, [[W, 1], [1, W]]), cidxf[:])
                    nc.sync.dma_start(bass.AP(o_d, 64, [[W, 1], [1, W]]), mask[:])
                    nc.sync.dma_start(bass.AP(o_d, 96, [[W, 1], [1, W]]), wexp[:])
                    nc.sync.dma_start(bass.AP(o_d, 128, [[8, 1], [1, 8]]), t8v[:])
                    nc.sync.dma_start(bass.AP(o_d, 140, [[1, 1], [1, 1]]), tau[:])
                    nc.sync.dma_start(bass.AP(o_d, 160, [[256, 1], [1, 256]]),
                                      grand[:, 0:256])
                else:
                    # ------------- Phase 9: circulant output -------------
                    with tc.tile_pool(name="ost", bufs=4) as ostp, \
                         tc.tile_pool(name="ops", bufs=2, space="PSUM") as opsp:
                        for u in range(NT):
                            ops = opsp.tile([128, C], f32, tag="o")
                            for p in range(33):
                                up = (u + p) % NT
                                nc.tensor.matmul(
                                    ops[:], G[:, 128 * p:128 * (p + 1)],
                                    VT[:, C * up:C * (up + 1)],
                                    start=(p == 0), stop=(p == 32))
                            og = ostp.tile([128, C], f32, tag="og")
                            nc.scalar.activation(og[:], ops[:], AF.Identity,
                                                 scale=wrecb[:, 0:1])
                            nc.sync.dma_start(o_d[128 * u:128 * (u + 1), :], og[:])

    nc.finalize()
    return nc


def _get_nc():
    if "nc" not in _CACHE:
        _CACHE["nc"] = _build()
    return _CACHE["nc"]


def kernel(queries, keys, values):
    from concourse import bass_utils

    nc = _get_nc()
    b, l, h, e = queries.shape
    assert (b, l, h, e) == (B, L, 8, 64)
    ident = np.eye(128, dtype=np.float32)
    in_maps = []
    for i in range(B):
        in_maps.append({
            "q": np.ascontiguousarray(queries[i].reshape(L, C), dtype=np.float32),
            "k": np.ascontiguousarray(keys[i].reshape(L, C), dtype=np.float32),
            "v": np.ascontiguousarray(values[i].reshape(L, C), dtype=np.float32),
            "ident": ident,
        })
    trace = os.environ.get("AC_TRACE", "0") == "1"
    res = bass_utils.run_bass_kernel_spmd(
        nc, in_maps, core_ids=list(range(B)), trace=trace)
    if res.exec_time_ns is not None:
        kernel.last_exec_time_ns = res.exec_time_ns
        print(f"[kernel] HW exec time: {res.exec_time_ns} ns", file=sys.stderr)
    out = np.stack([res.results[i]["o"].reshape(L, h, e) for i in range(B)])
    return out


kernel.last_exec_time_ns = None



# revision 17
# speedup vs baseline: 1.2369x; 1.2369x over previous
"""AutoCorrelation (Autoformer) Bass kernel for 8 trn2 NeuronCores.

Problem: B=8, L=4096, H=8, E=64, TOP_K=8.
Sharding: data-parallel over batch (core b handles batch b); the cross-batch
mean for top-k index selection is a [4096]-element AllReduce.

Per-core algorithm (batch slice q,k,v: [L=4096, C=512] fp32, C = H*E):
  1. q,k tile loads interleaved; TensorE transposes -> qT,kT [C, L] bf16
     (PSUM->SBUF copies split across Scalar and Vector engines). The first
     correlation group is woven into the load loop so the PE ramps while DMA
     streams; v is prefetched and cast to bf16 on DVE during the correlation.
  2. Block-Toeplitz correlation on PE (bf16): for block offset m in [0,32):
       D_m[i,j] = sum_{u,c} qT[c,128u+i] * kT[c, 128((u+m)%32)+j]
     accumulated in PSUM tiles [128, 512] (4 block offsets per tile).
  3. mean_value[128m+d] = sum_i [D_m|D_{m+1}][i, i+d]: diagonal sums extracted
     with a DRAM "skewed-pitch" bounce (write pitch 4224, read pitch 4225),
     reads and ones-matmul means software-pipelined into later corr groups.
  4. AllReduce mean_value over the 8 cores in 3 slices; the first two (and
     their slice-top-8) hide under the correlation. Top-8 merge by threshold:
     24 slice candidates, the 16 losers keep weight 0. Per-batch weights
     gathered by a wide iota-compare masked reduce, summed across partitions
     with a ones matmul; exp() folded into the PSUM drain, softmax
     normalization folded into the output copies as a 1/sum scale.
  5. Output = sum_j w_j * roll(v, -d_j) as a 33-block circulant matmul in
     bf16. G [128, 4224] built in SBUF by gpsimd local_scatter of the
     bf16 exp-weights at diagonal offsets computed on DVE.
"""

import os
import sys
import numpy as np

sys.path.insert(0, "/opt/trn_rl_repo")

L = 4096
C = 512  # H*E
B = 8
NT = 32  # L/128 tiles
TOPK = 8
SCALE = 1.0 / 512.0  # mean over H*E
GW = 4224  # 33*128 circulant block columns

_CACHE = {}


def _build():
    import concourse.bass as bass
    import concourse.tile as tile
    from concourse import bacc, mybir

    # Split-AllReduce path (AC_ARSPLIT=1) hides ~5us more under the corr
    # phase but showed a rare intermittent race (1-in-~7 NaN); default to
    # the proven single-AllReduce mid-section.
    AR_SPLIT = os.environ.get("AC_ARSPLIT", "0") == "1"
    assert not AR_SPLIT, "compact-16 output assumes exactly 8 candidates"

    f32 = mybir.dt.float32
    bf16 = mybir.dt.bfloat16
    i32 = mybir.dt.int32
    u32 = mybir.dt.uint32
    AF = mybir.ActivationFunctionType
    ALU = mybir.AluOpType

    nc = bacc.Bacc(
        "TRN2", target_bir_lowering=False, debug=False, num_devices=B,
    )

    q_d = nc.dram_tensor("q", [L, C], f32, kind="ExternalInput")
    k_d = nc.dram_tensor("k", [L, C], f32, kind="ExternalInput")
    v_d = nc.dram_tensor("v", [L, C], f32, kind="ExternalInput")
    ident_d = nc.dram_tensor("ident", [128, 128], f32, kind="ExternalInput")
    o_d = nc.dram_tensor("o", [L, C], f32, kind="ExternalOutput")

    # DRAM scratch
    ed_d = nc.dram_tensor("ed", [128 * GW + 128], f32, kind="Internal")
    bri_d = nc.dram_tensor("bri", [32], i32, kind="Internal")
    brf_d = nc.dram_tensor("brf", [32], f32, kind="Internal")
    ccin_d = nc.dram_tensor("ccin", [L], f32, kind="Internal")
    ccout_d = nc.dram_tensor("ccout", [L], f32, kind="Internal",
                             addr_space="Shared")

    with tile.TileContext(nc) as tc:
        with tc.tile_pool(name="const", bufs=1) as constp, \
             tc.tile_pool(name="mvp", bufs=1) as mvp, \
             tc.tile_pool(name="vtp", bufs=1) as vtp:
            ident = constp.tile([128, 128], f32)
            nc.sync.dma_start(ident[:], ident_d[:, :])
            identb = constp.tile([128, 128], bf16)
            nc.vector.tensor_copy(identb[:], ident[:])
            ones = constp.tile([128, 1], f32)
            nc.vector.memset(ones[:], SCALE)
            onesw = constp.tile([128, 1], f32)
            nc.vector.memset(onesw[:], 1.0)
            mv = mvp.tile([1, L], f32)
            # v tiles, bf16, tile-major, duplicated (blocks 0..31, 0..31) so
            # the output phase can take dynamic 512-col slices without mod-32
            # wraparound handling.
            VT = vtp.tile([128, 2 * NT * C], bf16)

            # W = number of top-k candidates carried through the mid-section.
            # With the split AllReduce, top-8 runs per slice (24 candidates);
            # losers are masked to weight 0 and scatter zeros into G.
            W = 24 if AR_SPLIT else 8

            # data-independent constants for the mid-section (built early,
            # off the critical path):
            # ioWf[p, 32j+f] = 32p + f  (weight-gather compare basis)
            ioW = constp.tile([128, 32 * W], i32)
            nc.gpsimd.iota(ioW[:], [[0, W], [1, 32]], base=0,
                           channel_multiplier=32)
            ioWf = constp.tile([128, 32 * W], f32)
            nc.vector.tensor_copy(ioWf[:], ioW[:])
            # tpW[p, j] = p  (partition index)
            tpW = constp.tile([128, W], i32)
            nc.gpsimd.iota(tpW[:], [[0, W]], base=0, channel_multiplier=1)
            # coff[p, W*c+j] = 1024c (local_scatter chunk offsets)
            coff = constp.tile([128, 2 * W], i32)
            nc.gpsimd.iota(coff[:], [[1024, 2], [0, W]], base=0,
                           channel_multiplier=0)
            # joff[p, j] = 256j (compact G16 slot-pair base columns)
            joff = constp.tile([128, W], i32)
            nc.gpsimd.iota(joff[:], [[256, W]], base=0,
                           channel_multiplier=0)
            # mid-section tiles that are produced during the corr phase
            grand = constp.tile([1, L], f32)
            cval = constp.tile([1, W], f32)
            cidxu = constp.tile([1, W], u32)
            cidx = constp.tile([1, W], i32)

            # -------- Phase 1+2: pipelined load/transpose/correlation -------
            # q,k tile loads interleaved; the first correlation group (mg=0)
            # is woven into the load loop so PE ramps while DMA streams; the
            # shear bounce + mean matmuls are interleaved into the mg loop.
            with tc.tile_pool(name="qkT", bufs=1) as qkTp, \
                 tc.tile_pool(name="ld", bufs=6) as ldp, \
                 tc.tile_pool(name="sh", bufs=4) as shp, \
                 tc.tile_pool(name="trps", bufs=4, space="PSUM") as trpsp, \
                 tc.tile_pool(name="corr", bufs=2, space="PSUM") as corrp, \
                 tc.tile_pool(name="mvps", bufs=2, space="PSUM") as mvpsp, \
                 tc.tile_pool(name="pp", bufs=2) as ppp:
                # q/k transposed bf16, group-major in one tile per tensor
                qTall = qkTp.tile([128, 4 * L], bf16, name="qTall")
                kTall = qkTp.tile([128, 4 * L], bf16, name="kTall")

                def corr_mms(P, mg, u, first):
                    s = 128 * ((u + mg) % NT)
                    last = (u == NT - 1)
                    for g in range(4):
                        lhsT = kTall[:, g * L + 128 * u:g * L + 128 * (u + 1)]
                        st = first and g == 0
                        lastg = last and g == 3
                        if s <= L - 512:
                            nc.tensor.matmul(
                                P[:, :], lhsT,
                                qTall[:, g * L + s:g * L + s + 512],
                                start=st, stop=lastg,
                                skip_group_check=True)
                        else:
                            n1 = L - s
                            nc.tensor.matmul(
                                P[:, 0:n1], lhsT,
                                qTall[:, g * L + s:g * L + L],
                                start=st, stop=False,
                                skip_group_check=True)
                            nc.tensor.matmul(
                                P[:, n1:512], lhsT,
                                qTall[:, g * L:g * L + 512 - n1],
                                start=st, stop=lastg,
                                skip_group_check=True)

                P0 = corrp.tile([128, 512], f32, tag="P", name="P0")
                P1 = corrp.tile([128, 512], f32, tag="P", name="P1")
                for u in range(NT):
                    for (src, dstT, ceng) in ((q_d, qTall, nc.scalar),
                                              (k_d, kTall, nc.vector)):
                        t_in = ldp.tile([128, C], f32, tag="ld")
                        nc.sync.dma_start(t_in[:], src[128 * u:128 * (u + 1), :])
                        # cast to bf16 on the opposite engine from the copies
                        # so the transposes run at 1 cycle/row instead of 2
                        t_b = ldp.tile([128, C], bf16, tag="ldb")
                        if ceng is nc.scalar:
                            nc.vector.tensor_copy(t_b[:], t_in[:])
                        else:
                            nc.scalar.activation(t_b[:], t_in[:], AF.Identity)
                        for g in range(4):
                            ps = trpsp.tile([128, 128], bf16, tag="tr")
                            nc.tensor.transpose(
                                ps[:], t_b[:, 128 * g:128 * (g + 1)], identb[:])
                            # PSUM->SBUF copy casts bf16 -> fp8e4
                            dsl = dstT[:, g * L + 128 * u:g * L + 128 * (u + 1)]
                            if ceng is nc.scalar:
                                nc.scalar.activation(dsl, ps[:], AF.Identity)
                            else:
                                nc.vector.tensor_copy(dsl, ps[:])
                    # weave corr groups mg=0 and mg=4 behind the loads: the
                    # window for (mg, u_w) needs q tiles u_w+mg/128..+3 and k
                    # tile u_w, all loaded by iteration u_w+mg/128+3.
                    if u >= 3 and u - 3 <= NT - 5:
                        corr_mms(P0, 0, u - 3, first=(u == 3))
                    if u >= 7 and u - 7 <= NT - 8:
                        corr_mms(P1, 4, u - 7, first=(u == 7))

                # v prefetch: DMA queued behind q/k, casts on DVE (written to
                # both duplicate halves); completes during the corr matmuls.
                for u in range(NT):
                    t_in = ldp.tile([128, C], f32, tag="vld")
                    nc.sync.dma_start(t_in[:], v_d[128 * u:128 * (u + 1), :])
                    nc.vector.tensor_copy(VT[:, C * u:C * (u + 1)], t_in[:])
                    nc.vector.tensor_copy(
                        VT[:, C * (NT + u):C * (NT + u + 1)], t_in[:])

                def drain_chunk(P, ch):
                    PPt = ppp.tile([128, 512], f32, tag="pp")
                    nc.scalar.activation(PPt[:], P[:], AF.Identity)
                    # shear-write this chunk (pitch GW=4224)
                    nc.sync.dma_start(
                        bass.AP(ed_d, 512 * ch, [[GW, 128], [1, 512]]),
                        PPt[:])
                    if ch == 0:
                        # wrap block: ed cols [4096:4224] = chunk0 cols [0:128]
                        nc.sync.dma_start(
                            bass.AP(ed_d, L, [[GW, 128], [1, 128]]),
                            PPt[:, 0:128])

                SHs = {}

                def shear_read(ch):
                    # skewed read: pitch GW+1 extracts diagonals; chunk ch
                    # touches write-chunks ch and ch+1 (skew <= 127), so this
                    # must be issued after drain_chunk(ch+1).
                    SH = shp.tile([128, 512], f32, tag="sh")
                    nc.sync.dma_start(
                        SH[:],
                        bass.AP(ed_d, 512 * ch, [[GW + 1, 128], [1, 512]]))
                    SHs[ch] = SH

                def mean_mm(ch):
                    mps = mvpsp.tile([1, 512], f32, tag="mv")
                    nc.tensor.matmul(
                        mps[:], ones[:], SHs[ch][:], start=True, stop=True)
                    nc.scalar.activation(
                        mv[:, 512 * ch:512 * (ch + 1)], mps[:], AF.Identity)

                # finish mg=0 (wrap rows), then the remaining 7 groups.
                # Shear chunk ch-1 is read back right after drain(ch) lands;
                # its mean-matmul runs mid-way through the NEXT group so it
                # never heads the PE queue while the DMA is in flight. The
                # first AllReduce half launches once mean chunks 0..3 are
                # done and hides under the remaining corr groups.
                def ar_slice(lo, hi):
                    nc.sync.dma_start(ccin_d[lo:hi], mv[:, lo:hi])
                    nc.gpsimd.collective_compute(
                        "AllReduce", mybir.AluOpType.add,
                        replica_groups=[list(range(B))],
                        ins=[ccin_d[lo:hi]], outs=[ccout_d[lo:hi]],
                    )

                def slice_topk(lo, hi, slot):
                    # local top-8 of grand[lo:hi]; indices made global
                    nc.sync.dma_start(grand[:, lo:hi], ccout_d[lo:hi])
                    s = slice(8 * slot, 8 * slot + 8)
                    nc.vector.max_with_indices(cval[:, s], cidxu[:, s],
                                               grand[:, lo:hi])
                    nc.vector.tensor_scalar(cidx[:, s], cidxu[:, s], lo, None,
                                            ALU.add)

                for u in range(NT - 4, NT):
                    corr_mms(P0, 0, u, first=False)
                drain_chunk(P0, 0)
                for u in range(NT - 7, NT):
                    corr_mms(P1, 4, u, first=False)
                drain_chunk(P1, 1)
                shear_read(0)
                for mg in range(8, NT, 4):
                    P = corrp.tile([128, 512], f32, tag="P")
                    ch = mg // 4
                    for u in range(NT):
                        corr_mms(P, mg, u, first=(u == 0))
                        if u == 16 and ch >= 2:
                            mean_mm(ch - 2)
                        if AR_SPLIT and u == 24 and ch == 5:
                            ar_slice(0, 2048)       # hidden under corr
                        if AR_SPLIT and u == 8 and ch == 7:
                            slice_topk(0, 2048, 0)  # hidden under corr
                        if AR_SPLIT and u == 24 and ch == 7:
                            ar_slice(2048, 3072)    # hidden under corr tail
                    drain_chunk(P, ch)
                    shear_read(ch - 1)
                shear_read(7)
                if AR_SPLIT:
                    slice_topk(2048, 3072, 1)
                mean_mm(6)
                mean_mm(7)

            # ------------- Phase 5: final collective slice -------------
            if AR_SPLIT:
                nc.sync.dma_start(ccin_d[3072:L], mv[:, 3072:L])
                nc.gpsimd.collective_compute(
                    "AllReduce", mybir.AluOpType.add,
                    replica_groups=[list(range(B))],
                    ins=[ccin_d[3072:L]], outs=[ccout_d[3072:L]],
                )
            else:
                nc.sync.dma_start(ccin_d[:], mv[:])
                nc.gpsimd.collective_compute(
                    "AllReduce", mybir.AluOpType.add,
                    replica_groups=[list(range(B))],
                    ins=[ccin_d[:]], outs=[ccout_d[:]],
                )
            with tc.tile_pool(name="small", bufs=1) as sp:
                # per-batch mean reshaped [128, 32] for the weight gather
                # (local values only)
                mv2 = sp.tile([128, 32], f32)
                nc.sync.dma_start(mv2[:], bass.AP(ccin_d, 0, [[32, 128], [1, 32]]))

                # ---------------- Phase 6: top-8 ----------------
                if AR_SPLIT:
                    # last slice top-8, then threshold-merge the 24
                    # candidates: winners are the global top-8; losers keep
                    # weight 0 downstream.
                    nc.sync.dma_start(grand[:, 3072:L], ccout_d[3072:L])
                    nc.vector.max_with_indices(cval[:, 16:24], cidxu[:, 16:24],
                                               grand[:, 3072:L])
                    nc.vector.tensor_scalar(cidx[:, 16:24], cidxu[:, 16:24],
                                            3072, None, ALU.add)
                    t8v = sp.tile([1, 8], f32)
                    t8i = sp.tile([1, 8], u32)
                    nc.vector.max_with_indices(t8v[:], t8i[:], cval[:])
                    tau = sp.tile([1, 1], f32)
                    nc.vector.tensor_reduce(tau[:], t8v[:],
                                            mybir.AxisListType.X, ALU.min)
                    mask = sp.tile([1, W], f32)
                    nc.vector.tensor_tensor(
                        mask[:], cval[:],
                        bass.AP(tau.tensor, 0, [[1, 1], [0, W]]), ALU.is_ge)
                else:
                    nc.sync.dma_start(grand[:], ccout_d[:])
                    nc.vector.max_with_indices(cval[:], cidxu[:], grand[:])
                    nc.vector.tensor_copy(cidx[:], cidxu[:])

                # ------------- Phase 7: weights + softmax -------------
                BOUNCE_BC = os.environ.get("AC_BCB", "0") == "1"
                idxb = sp.tile([128, W], i32)
                if BOUNCE_BC:
                    # partition broadcast via DRAM bounce (stride-0 read)
                    nc.sync.dma_start(bri_d[0:W], cidx[:])
                    nc.sync.dma_start(idxb[:],
                                      bass.AP(bri_d, 0, [[0, 128], [1, W]]))
                else:
                    nc.gpsimd.partition_broadcast(idxb[:], cidx[:],
                                                  channels=128)
                idxf = sp.tile([128, W], f32)
                nc.vector.tensor_copy(idxf[:], idxb[:])

                # wpW[p, j] = sum_f [32p+f == d_j] * mv2[p, f] in one wide
                # masked-reduce over [128, W, 32] broadcast access patterns.
                def ap3(t, d):
                    return bass.AP(t.tensor, 0, [[t.shape[1], 128]] + d)
                cmpm = sp.tile([128, 32 * W], f32)
                nc.vector.tensor_tensor(
                    ap3(cmpm, [[32, W], [1, 32]]),
                    ap3(ioWf, [[32, W], [1, 32]]),
                    ap3(idxf, [[1, W], [0, 32]]), ALU.is_equal)
                prods = sp.tile([128, 32 * W], f32)
                nc.vector.tensor_tensor(
                    ap3(prods, [[32, W], [1, 32]]),
                    ap3(cmpm, [[32, W], [1, 32]]),
                    ap3(mv2, [[0, W], [1, 32]]), ALU.mult)
                wpW = sp.tile([128, W], f32)
                nc.vector.tensor_reduce(
                    wpW[:], ap3(prods, [[32, W], [1, 32]]),
                    mybir.AxisListType.X, ALU.add)
                # cross-partition sum via ones matmul; Exp folded into the
                # PSUM drain; non-top-8 candidates masked to 0. Softmax
                # normalization is folded into the output copies (scale by
                # 1/sum), so G holds unnormalized exp(w).
                with tc.tile_pool(name="wps", bufs=1, space="PSUM") as wpsp:
                    wps = wpsp.tile([1, W], f32)
                    nc.tensor.matmul(wps[:], onesw[:], wpW[:],
                                     start=True, stop=True)
                    wexp = sp.tile([1, W], f32)
                    nc.scalar.activation(wexp[:], wps[:], AF.Exp)
                if AR_SPLIT:
                    nc.vector.tensor_tensor(wexp[:], wexp[:], mask[:],
                                            ALU.mult)
                wsum = sp.tile([1, 1], f32)
                nc.vector.tensor_reduce(wsum[:], wexp[:], mybir.AxisListType.X,
                                        ALU.add)
                wrec = sp.tile([1, 1], f32)
                nc.vector.reciprocal(wrec[:], wsum[:])
                wrecb = sp.tile([128, 1], f32)
                if BOUNCE_BC:
                    nc.sync.dma_start(brf_d[0:W], wexp[:])
                    nc.sync.dma_start(brf_d[W:W + 1], wrec[:])
                    nc.sync.dma_start(wrecb[:],
                                      bass.AP(brf_d, W, [[0, 128], [1, 1]]))
                else:
                    nc.gpsimd.partition_broadcast(wrecb[:], wrec[:],
                                                  channels=128)

                # -------- Phase 8: G16 diag offsets + local_scatter --------
                # Compact slot layout: candidate j owns slots (2j, 2j+1) =
                # source blocks (b_j, b_j+1); within the 256-col slot pair the
                # scatter position is ((p - r_j) & 255), identical banded
                # semantics to the full circulant but only 16 blocks.
                # offc[p,j] = 256*j + ((p - (d & 127)) & 255)
                rj = sp.tile([128, W], i32)
                nc.vector.tensor_scalar(rj[:], idxb[:], 127, None,
                                        ALU.bitwise_and)
                tmr = sp.tile([128, W], i32)
                nc.vector.tensor_tensor(tmr[:], tpW[:], rj[:], ALU.subtract)
                nc.vector.tensor_scalar(tmr[:], tmr[:], 255, None,
                                        ALU.bitwise_and)
                offc = sp.tile([128, W], i32)
                nc.vector.tensor_tensor(offc[:], joff[:], tmr[:], ALU.add)
                # per-candidate source block index for the dynamic VT slices
                blk = sp.tile([1, W], i32)
                nc.vector.tensor_scalar(blk[:], cidx[:], 7, None,
                                        ALU.arith_shift_right)

                wvals = sp.tile([128, W], f32)
                if BOUNCE_BC:
                    nc.sync.dma_start(wvals[:],
                                      bass.AP(brf_d, 0, [[0, 128], [1, W]]))
                else:
                    nc.gpsimd.partition_broadcast(wvals[:], wexp[:],
                                                  channels=128)
                wb = sp.tile([128, W], bf16)
                nc.vector.tensor_copy(wb[:], wvals[:])

                # local_scatter builds G16 [128, 16*128] in SBUF (zeroes dst,
                # then dst[p, idx[p,j]] = wb[p,j]); 2 chunks of 1024 columns
                # to fit the GPSIMD local-RAM limit; out-of-chunk indices go
                # negative (ignored).
                G16 = sp.tile([128, 2048], bf16)
                i16 = mybir.dt.int16
                tall = sp.tile([128, 2 * W], i32)
                nc.vector.tensor_tensor(
                    ap3(tall, [[W, 2], [1, W]]),
                    ap3(offc, [[0, 2], [1, W]]),
                    ap3(coff, [[W, 2], [1, W]]), ALU.subtract)
                gall = sp.tile([128, 2 * W], i32)
                nc.vector.tensor_scalar(gall[:], tall[:], 1024, None,
                                        ALU.is_ge)
                nc.vector.tensor_scalar(gall[:], gall[:], 8192, None,
                                        ALU.mult)
                nc.vector.tensor_tensor(tall[:], tall[:], gall[:],
                                        ALU.subtract)
                idx16 = sp.tile([128, 2 * W], i16)
                nc.vector.tensor_copy(idx16[:], tall[:])
                for c in range(2):
                    nc.gpsimd.local_scatter(
                        G16[:, 1024 * c:1024 * (c + 1)], wb[:],
                        idx16[:, W * c:W * (c + 1)],
                        channels=128, num_elems=1024, num_idxs=W)
                # source-block indices -> PE sequencer registers
                svals = [
                    nc.values_load(
                        blk[:, j:j + 1], engines=[mybir.EngineType.PE],
                        min_val=0, max_val=31,
                        skip_runtime_bounds_check=True)
                    for j in range(TOPK)
                ]

                if os.environ.get("AC_DBG", "0") == "1" and AR_SPLIT:
                    cidxf = sp.tile([1, W], f32)
                    nc.vector.tensor_copy(cidxf[:], cidx[:])
                    nc.sync.dma_start(bass.AP(o_d, 0, [[W, 1], [1, W]]), cval[:])
                    nc.sync.dma_start(bass.AP(o_d, 32, [[W, 1], [1, W]]), cidxf[:])
                    nc.sync.dma_start(bass.AP(o_d, 64, [[W, 1], [1, W]]), mask[:])
                    nc.sync.dma_start(bass.AP(o_d, 96, [[W, 1], [1, W]]), wexp[:])
                    nc.sync.dma_start(bass.AP(o_d, 128, [[8, 1], [1, 8]]), t8v[:])
                    nc.sync.dma_start(bass.AP(o_d, 140, [[1, 1], [1, 1]]), tau[:])
                    nc.sync.dma_start(bass.AP(o_d, 160, [[256, 1], [1, 256]]),
                                      grand[:, 0:256])
                else:
                    # --------- Phase 9: compact dynamic-block output ---------
                    # 16 matmuls per tile: candidate j's slot pair (2j, 2j+1)
                    # contracts against VT blocks (b_j+u) and (b_j+u+1), whose
                    # offsets come from PE registers (no mod: VT duplicated).
                    with tc.tile_pool(name="ost", bufs=4) as ostp, \
                         tc.tile_pool(name="ops", bufs=2, space="PSUM") as opsp:
                        for u in range(NT):
                            ops = opsp.tile([128, C], f32, tag="o")
                            for j in range(TOPK):
                                for h in range(2):
                                    sl = 2 * j + h
                                    rhs = VT[:, bass.ds(
                                        (svals[j] + (u + h)) * C, C)]
                                    nc.tensor.matmul(
                                        ops[:],
                                        G16[:, 128 * sl:128 * (sl + 1)],
                                        rhs,
                                        start=(sl == 0), stop=(sl == 15),
                                        skip_group_check=True)
                            og = ostp.tile([128, C], f32, tag="og")
                            nc.scalar.activation(og[:], ops[:], AF.Identity,
                                                 scale=wrecb[:, 0:1])
                            nc.sync.dma_start(o_d[128 * u:128 * (u + 1), :], og[:])

    nc.finalize()
    return nc


def _get_nc():
    if "nc" not in _CACHE:
        _CACHE["nc"] = _build()
    return _CACHE["nc"]


def kernel(queries, keys, values):
    from concourse import bass_utils

    nc = _get_nc()
    b, l, h, e = queries.shape
    assert (b, l, h, e) == (B, L, 8, 64)
    ident = np.eye(128, dtype=np.float32)
    in_maps = []
    for i in range(B):
        in_maps.append({
            "q": np.ascontiguousarray(queries[i].reshape(L, C), dtype=np.float32),
            "k": np.ascontiguousarray(keys[i].reshape(L, C), dtype=np.float32),
            "v": np.ascontiguousarray(values[i].reshape(L, C), dtype=np.float32),
            "ident": ident,
        })
    trace = os.environ.get("AC_TRACE", "0") == "1"
    res = bass_utils.run_bass_kernel_spmd(
        nc, in_maps, core_ids=list(range(B)), trace=trace)
    if res.exec_time_ns is not None:
        kernel.last_exec_time_ns = res.exec_time_ns
        print(f"[kernel] HW exec time: {res.exec_time_ns} ns", file=sys.stderr)
    out = np.stack([res.results[i]["o"].reshape(L, h, e) for i in range(B)])
    return out


kernel.last_exec_time_ns = None



# revision 27
# speedup vs baseline: 1.2427x; 1.0047x over previous
"""AutoCorrelation (Autoformer) Bass kernel for 8 trn2 NeuronCores.

Problem: B=8, L=4096, H=8, E=64, TOP_K=8.
Sharding: data-parallel over batch (core b handles batch b); the cross-batch
mean for top-k index selection is a [4096]-element AllReduce.

Per-core algorithm (batch slice q,k,v: [L=4096, C=512] fp32, C = H*E):
  1. q,k tile loads interleaved; TensorE transposes -> qT,kT [C, L] bf16
     (PSUM->SBUF copies split across Scalar and Vector engines). The first
     correlation group is woven into the load loop so the PE ramps while DMA
     streams; v is prefetched and cast to bf16 on DVE during the correlation.
  2. Block-Toeplitz correlation on PE (bf16): for block offset m in [0,32):
       D_m[i,j] = sum_{u,c} qT[c,128u+i] * kT[c, 128((u+m)%32)+j]
     accumulated in PSUM tiles [128, 512] (4 block offsets per tile).
  3. mean_value[128m+d] = sum_i [D_m|D_{m+1}][i, i+d]: diagonal sums extracted
     with a DRAM "skewed-pitch" bounce (write pitch 4224, read pitch 4225),
     reads and ones-matmul means software-pipelined into later corr groups.
  4. AllReduce mean_value over the 8 cores in 3 slices; the first two (and
     their slice-top-8) hide under the correlation. Top-8 merge by threshold:
     24 slice candidates, the 16 losers keep weight 0. Per-batch weights
     gathered by a wide iota-compare masked reduce, summed across partitions
     with a ones matmul; exp() folded into the PSUM drain, softmax
     normalization folded into the output copies as a 1/sum scale.
  5. Output = sum_j w_j * roll(v, -d_j) as a 33-block circulant matmul in
     bf16. G [128, 4224] built in SBUF by gpsimd local_scatter of the
     bf16 exp-weights at diagonal offsets computed on DVE.
"""

import os
import sys
import numpy as np

sys.path.insert(0, "/opt/trn_rl_repo")

L = 4096
C = 512  # H*E
B = 8
NT = 32  # L/128 tiles
TOPK = 8
SCALE = 1.0 / 512.0  # mean over H*E
GW = 4224  # 33*128 circulant block columns

_CACHE = {}


def _build():
    import concourse.bass as bass
    import concourse.tile as tile
    from concourse import bacc, mybir

    f32 = mybir.dt.float32
    bf16 = mybir.dt.bfloat16
    i32 = mybir.dt.int32
    u32 = mybir.dt.uint32
    AF = mybir.ActivationFunctionType
    ALU = mybir.AluOpType

    nc = bacc.Bacc(
        "TRN2", target_bir_lowering=False, debug=False, num_devices=B,
    )

    q_d = nc.dram_tensor("q", [L, C], f32, kind="ExternalInput")
    k_d = nc.dram_tensor("k", [L, C], f32, kind="ExternalInput")
    v_d = nc.dram_tensor("v", [L, C], f32, kind="ExternalInput")
    ident_d = nc.dram_tensor("ident", [128, 128], f32, kind="ExternalInput")
    o_d = nc.dram_tensor("o", [L, C], f32, kind="ExternalOutput")

    # DRAM scratch
    ed_d = nc.dram_tensor("ed", [128 * GW + 128], f32, kind="Internal")
    bri_d = nc.dram_tensor("bri", [32], i32, kind="Internal")
    brf_d = nc.dram_tensor("brf", [32], f32, kind="Internal")
    ccin_d = nc.dram_tensor("ccin", [L], f32, kind="Internal")
    ccout_d = nc.dram_tensor("ccout", [L], f32, kind="Internal",
                             addr_space="Shared")
    cand_d = nc.dram_tensor("cand", [1024], f32, kind="Internal")
    bar_i = nc.dram_tensor("bar_i", [1], mybir.dt.uint8, kind="Internal")
    bar_o = nc.dram_tensor("bar_o", [B], mybir.dt.uint8, kind="Internal")

    with tile.TileContext(nc) as tc:
        with tc.tile_pool(name="const", bufs=1) as constp, \
             tc.tile_pool(name="mvp", bufs=1) as mvp, \
             tc.tile_pool(name="vtp", bufs=1) as vtp:
            ident = constp.tile([128, 128], f32)
            nc.sync.dma_start(ident[:], ident_d[:, :])
            identb = constp.tile([128, 128], bf16)
            nc.vector.tensor_copy(identb[:], ident[:])
            ones = constp.tile([128, 1], f32)
            nc.vector.memset(ones[:], SCALE)
            onesw = constp.tile([128, 1], f32)
            nc.vector.memset(onesw[:], 1.0)
            mv = mvp.tile([1, L], f32)
            # v tiles, bf16, tile-major, duplicated (blocks 0..31, 0..31) so
            # the output phase can take dynamic 512-col slices without mod-32
            # wraparound handling.
            VT = vtp.tile([128, 2 * NT * C], bf16)

            W = 8  # top-k candidates carried through the mid-section

            # data-independent constants for the mid-section (built early,
            # off the critical path):
            # ioWf[p, 32j+f] = 32p + f  (weight-gather compare basis)
            ioW = constp.tile([128, 32 * W], i32)
            nc.gpsimd.iota(ioW[:], [[0, W], [1, 32]], base=0,
                           channel_multiplier=32)
            ioWf = constp.tile([128, 32 * W], f32)
            nc.vector.tensor_copy(ioWf[:], ioW[:])
            # tpW[p, j] = p  (partition index)
            tpW = constp.tile([128, W], i32)
            nc.gpsimd.iota(tpW[:], [[0, W]], base=0, channel_multiplier=1)
            # coff[p, W*c+j] = 1024c (local_scatter chunk offsets)
            coff = constp.tile([128, 2 * W], i32)
            nc.gpsimd.iota(coff[:], [[1024, 2], [0, W]], base=0,
                           channel_multiplier=0)
            # joff[p, j] = 256j (compact G16 slot-pair base columns)
            joff = constp.tile([128, W], i32)
            nc.gpsimd.iota(joff[:], [[256, W]], base=0,
                           channel_multiplier=0)
            # io32[p, c] = 32p + c (global delay index for the enc-topk)
            io32 = constp.tile([128, 32], i32)
            nc.gpsimd.iota(io32[:], [[1, 32]], base=0, channel_multiplier=32)
            # mid-section tiles that are produced during the corr phase
            cidx = constp.tile([1, W], i32)

            # warm-up AllGather: spins up the ncfw collective pipeline during
            # the load phase so the mid-section AllReduce slices skip the
            # ~12us cold-launch latency.
            nc.gpsimd.collective_compute(
                "AllGather", mybir.AluOpType.bypass,
                replica_groups=[list(range(B))],
                ins=[bar_i[:]], outs=[bar_o[:]])

            # -------- Phase 1+2: pipelined load/transpose/correlation -------
            # q,k tile loads interleaved; the first correlation group (mg=0)
            # is woven into the load loop so PE ramps while DMA streams; the
            # shear bounce + mean matmuls are interleaved into the mg loop.
            with tc.tile_pool(name="qkT", bufs=1) as qkTp, \
                 tc.tile_pool(name="ld", bufs=6) as ldp, \
                 tc.tile_pool(name="sh", bufs=4) as shp, \
                 tc.tile_pool(name="trps", bufs=4, space="PSUM") as trpsp, \
                 tc.tile_pool(name="corr", bufs=2, space="PSUM") as corrp, \
                 tc.tile_pool(name="mvps", bufs=2, space="PSUM") as mvpsp, \
                 tc.tile_pool(name="pp", bufs=2) as ppp:
                # q/k transposed bf16, group-major in one tile per tensor
                qTall = qkTp.tile([128, 4 * L], bf16, name="qTall")
                kTall = qkTp.tile([128, 4 * L], bf16, name="kTall")

                def corr_mms(P, mg, u, first):
                    s = 128 * ((u + mg) % NT)
                    last = (u == NT - 1)
                    for g in range(4):
                        lhsT = kTall[:, g * L + 128 * u:g * L + 128 * (u + 1)]
                        st = first and g == 0
                        lastg = last and g == 3
                        if s <= L - 512:
                            nc.tensor.matmul(
                                P[:, :], lhsT,
                                qTall[:, g * L + s:g * L + s + 512],
                                start=st, stop=lastg,
                                skip_group_check=True)
                        else:
                            n1 = L - s
                            nc.tensor.matmul(
                                P[:, 0:n1], lhsT,
                                qTall[:, g * L + s:g * L + L],
                                start=st, stop=False,
                                skip_group_check=True)
                            nc.tensor.matmul(
                                P[:, n1:512], lhsT,
                                qTall[:, g * L:g * L + 512 - n1],
                                start=st, stop=lastg,
                                skip_group_check=True)

                P0 = corrp.tile([128, 512], f32, tag="P", name="P0")
                P1 = corrp.tile([128, 512], f32, tag="P", name="P1")
                for u in range(NT):
                    for (src, dstT, ceng) in ((q_d, qTall, nc.scalar),
                                              (k_d, kTall, nc.vector)):
                        t_in = ldp.tile([128, C], f32, tag="ld")
                        nc.sync.dma_start(t_in[:], src[128 * u:128 * (u + 1), :])
                        # cast to bf16 on the opposite engine from the copies
                        # so the transposes run at 1 cycle/row instead of 2
                        t_b = ldp.tile([128, C], bf16, tag="ldb")
                        if ceng is nc.scalar:
                            nc.vector.tensor_copy(t_b[:], t_in[:])
                        else:
                            nc.scalar.activation(t_b[:], t_in[:], AF.Identity)
                        for g in range(4):
                            ps = trpsp.tile([128, 128], bf16, tag="tr")
                            nc.tensor.transpose(
                                ps[:], t_b[:, 128 * g:128 * (g + 1)], identb[:])
                            # PSUM->SBUF copy casts bf16 -> fp8e4
                            dsl = dstT[:, g * L + 128 * u:g * L + 128 * (u + 1)]
                            if ceng is nc.scalar:
                                nc.scalar.activation(dsl, ps[:], AF.Identity)
                            else:
                                nc.vector.tensor_copy(dsl, ps[:])
                    # weave corr groups mg=0 and mg=4 behind the loads: the
                    # window for (mg, u_w) needs q tiles u_w+mg/128..+3 and k
                    # tile u_w, all loaded by iteration u_w+mg/128+3.
                    if u >= 3 and u - 3 <= NT - 5:
                        corr_mms(P0, 0, u - 3, first=(u == 3))
                    if u >= 7 and u - 7 <= NT - 8:
                        corr_mms(P1, 4, u - 7, first=(u == 7))

                # v prefetch: DMA queued behind q/k, casts on DVE (written to
                # both duplicate halves); completes during the corr matmuls.
                for u in range(NT):
                    t_in = ldp.tile([128, C], f32, tag="vld")
                    nc.sync.dma_start(t_in[:], v_d[128 * u:128 * (u + 1), :])
                    nc.vector.tensor_copy(VT[:, C * u:C * (u + 1)], t_in[:])
                    nc.vector.tensor_copy(
                        VT[:, C * (NT + u):C * (NT + u + 1)], t_in[:])

                def drain_chunk(P, ch):
                    PPt = ppp.tile([128, 512], f32, tag="pp")
                    nc.scalar.activation(PPt[:], P[:], AF.Identity)
                    # shear-write this chunk (pitch GW=4224)
                    nc.sync.dma_start(
                        bass.AP(ed_d, 512 * ch, [[GW, 128], [1, 512]]),
                        PPt[:])
                    if ch == 0:
                        # wrap block: ed cols [4096:4224] = chunk0 cols [0:128]
                        nc.sync.dma_start(
                            bass.AP(ed_d, L, [[GW, 128], [1, 128]]),
                            PPt[:, 0:128])

                SHs = {}

                def shear_read(ch):
                    # skewed read: pitch GW+1 extracts diagonals; chunk ch
                    # touches write-chunks ch and ch+1 (skew <= 127), so this
                    # must be issued after drain_chunk(ch+1).
                    SH = shp.tile([128, 512], f32, tag="sh")
                    nc.sync.dma_start(
                        SH[:],
                        bass.AP(ed_d, 512 * ch, [[GW + 1, 128], [1, 512]]))
                    SHs[ch] = SH

                def mean_mm(ch):
                    mps = mvpsp.tile([1, 512], f32, tag="mv")
                    nc.tensor.matmul(
                        mps[:], ones[:], SHs[ch][:], start=True, stop=True)
                    nc.scalar.activation(
                        mv[:, 512 * ch:512 * (ch + 1)], mps[:], AF.Identity)

                # finish mg=0 (wrap rows), then the remaining 7 groups.
                # Shear chunk ch-1 is read back right after drain(ch) lands;
                # its mean-matmul runs mid-way through the NEXT group so it
                # never heads the PE queue while the DMA is in flight.
                for u in range(NT - 4, NT):
                    corr_mms(P0, 0, u, first=False)
                drain_chunk(P0, 0)
                for u in range(NT - 7, NT):
                    corr_mms(P1, 4, u, first=False)
                drain_chunk(P1, 1)
                shear_read(0)
                for mg in range(8, NT, 4):
                    P = corrp.tile([128, 512], f32, tag="P")
                    ch = mg // 4
                    for u in range(NT):
                        corr_mms(P, mg, u, first=(u == 0))
                        if u == 16 and ch >= 2:
                            mean_mm(ch - 2)
                        if u == 24 and ch == 5:
                            # slice A (delays 0..2047, mean chunks 0..3) of
                            # the AllReduce launches mid-corr and hides fully
                            nc.sync.dma_start(ccin_d[0:2048], mv[:, 0:2048])
                            nc.gpsimd.collective_compute(
                                "AllReduce", mybir.AluOpType.add,
                                replica_groups=[list(range(B))],
                                ins=[ccin_d[0:2048]], outs=[ccout_d[0:2048]],
                            )
                    drain_chunk(P, ch)
                    shear_read(ch - 1)
                shear_read(7)
                mean_mm(6)
                mean_mm(7)

            # ------- Phase 5: AllReduce slice B (delays 2048..4095) -------
            nc.sync.dma_start(ccin_d[2048:L], mv[:, 2048:L])
            nc.gpsimd.collective_compute(
                "AllReduce", mybir.AluOpType.add,
                replica_groups=[list(range(B))],
                ins=[ccin_d[2048:L]], outs=[ccout_d[2048:L]],
            )
            with tc.tile_pool(name="small", bufs=1) as sp:
                # per-batch mean reshaped [128, 32] for the weight gather
                # (local values only)
                mv2 = sp.tile([128, 32], f32)
                nc.sync.dma_start(mv2[:], bass.AP(ccin_d, 0, [[32, 128], [1, 32]]))

                # ---------------- Phase 6: top-8 ----------------
                # delay d lives at [partition d>>5, col d&31]; slice A covers
                # partitions 0..63, slice B 64..127. Select top-8 of the
                # summed mean via an encoded (value<<12 | delay) per-partition
                # max8 + flat 1024 rescan.
                grand2 = sp.tile([128, 32], f32)
                nc.sync.dma_start(grand2[0:64, :],
                                  bass.AP(ccout_d, 0, [[32, 64], [1, 32]]))
                nc.sync.dma_start(grand2[64:128, :],
                                  bass.AP(ccout_d, 2048, [[32, 64], [1, 32]]))
                encf = sp.tile([128, 32], f32)
                nc.vector.tensor_scalar(encf[:], grand2[:], 32.0, 2048.0,
                                        ALU.mult, ALU.add)
                enci = sp.tile([128, 32], i32)
                nc.vector.tensor_copy(enci[:], encf[:])  # rounds
                nc.vector.tensor_scalar(enci[:], enci[:], 4096, None,
                                        ALU.mult)
                nc.vector.tensor_tensor(enci[:], enci[:], io32[:], ALU.add)
                nc.vector.tensor_copy(encf[:], enci[:])  # exact (< 2^24)
                c8 = sp.tile([128, 8], f32)
                c8i = sp.tile([128, 8], u32)
                nc.vector.max_with_indices(c8[:], c8i[:], encf[:])
                nc.sync.dma_start(cand_d[:], c8[:])
                cflat = sp.tile([1, 1024], f32)
                nc.sync.dma_start(cflat[:], bass.AP(cand_d, 0, [[1024, 1], [1, 1024]]))
                t8 = sp.tile([1, 8], f32)
                t8i = sp.tile([1, 8], u32)
                nc.vector.max_with_indices(t8[:], t8i[:], cflat[:])
                enc8 = sp.tile([1, 8], i32)
                nc.vector.tensor_copy(enc8[:], t8[:])
                nc.vector.tensor_scalar(cidx[:], enc8[:], 4095, None,
                                        ALU.bitwise_and)

                # ------------- Phase 7: weights + softmax -------------
                BOUNCE_BC = os.environ.get("AC_BCB", "1") == "1"
                idxb = sp.tile([128, W], i32)
                if BOUNCE_BC:
                    # partition broadcast via DRAM bounce (stride-0 read)
                    nc.sync.dma_start(bri_d[0:W], cidx[:])
                    nc.sync.dma_start(idxb[:],
                                      bass.AP(bri_d, 0, [[0, 128], [1, W]]))
                else:
                    nc.gpsimd.partition_broadcast(idxb[:], cidx[:],
                                                  channels=128)
                idxf = sp.tile([128, W], f32)
                nc.vector.tensor_copy(idxf[:], idxb[:])

                # wpW[p, j] = sum_f [32p+f == d_j] * mv2[p, f] in one wide
                # masked-reduce over [128, W, 32] broadcast access patterns.
                def ap3(t, d):
                    return bass.AP(t.tensor, 0, [[t.shape[1], 128]] + d)
                cmpm = sp.tile([128, 32 * W], f32)
                nc.vector.tensor_tensor(
                    ap3(cmpm, [[32, W], [1, 32]]),
                    ap3(ioWf, [[32, W], [1, 32]]),
                    ap3(idxf, [[1, W], [0, 32]]), ALU.is_equal)
                prods = sp.tile([128, 32 * W], f32)
                nc.vector.tensor_tensor(
                    ap3(prods, [[32, W], [1, 32]]),
                    ap3(cmpm, [[32, W], [1, 32]]),
                    ap3(mv2, [[0, W], [1, 32]]), ALU.mult)
                wpW = sp.tile([128, W], f32)
                nc.vector.tensor_reduce(
                    wpW[:], ap3(prods, [[32, W], [1, 32]]),
                    mybir.AxisListType.X, ALU.add)
                # cross-partition sum via ones matmul; Exp folded into the
                # PSUM drain; non-top-8 candidates masked to 0. Softmax
                # normalization is folded into the output copies (scale by
                # 1/sum), so G holds unnormalized exp(w).
                with tc.tile_pool(name="wps", bufs=1, space="PSUM") as wpsp:
                    wps = wpsp.tile([1, W], f32)
                    nc.tensor.matmul(wps[:], onesw[:], wpW[:],
                                     start=True, stop=True)
                    wexp = sp.tile([1, W], f32)
                    nc.scalar.activation(wexp[:], wps[:], AF.Exp)
                wsum = sp.tile([1, 1], f32)
                nc.vector.tensor_reduce(wsum[:], wexp[:], mybir.AxisListType.X,
                                        ALU.add)
                wrec = sp.tile([1, 1], f32)
                nc.vector.reciprocal(wrec[:], wsum[:])
                wrecb = sp.tile([128, 1], f32)
                if BOUNCE_BC:
                    nc.sync.dma_start(brf_d[0:W], wexp[:])
                    nc.sync.dma_start(brf_d[W:W + 1], wrec[:])
                    nc.sync.dma_start(wrecb[:],
                                      bass.AP(brf_d, W, [[0, 128], [1, 1]]))
                else:
                    nc.gpsimd.partition_broadcast(wrecb[:], wrec[:],
                                                  channels=128)

                # -------- Phase 8: G16 diag offsets + local_scatter --------
                # Compact slot layout: candidate j owns slots (2j, 2j+1) =
                # source blocks (b_j, b_j+1); within the 256-col slot pair the
                # scatter position is ((p - r_j) & 255), identical banded
                # semantics to the full circulant but only 16 blocks.
                # offc[p,j] = 256*j + ((p - (d & 127)) & 255)
                rj = sp.tile([128, W], i32)
                nc.vector.tensor_scalar(rj[:], idxb[:], 127, None,
                                        ALU.bitwise_and)
                tmr = sp.tile([128, W], i32)
                nc.vector.tensor_tensor(tmr[:], tpW[:], rj[:], ALU.subtract)
                nc.vector.tensor_scalar(tmr[:], tmr[:], 255, None,
                                        ALU.bitwise_and)
                offc = sp.tile([128, W], i32)
                nc.vector.tensor_tensor(offc[:], joff[:], tmr[:], ALU.add)
                # per-candidate source block index for the dynamic VT slices
                blk = sp.tile([1, W], i32)
                nc.vector.tensor_scalar(blk[:], cidx[:], 7, None,
                                        ALU.arith_shift_right)

                wvals = sp.tile([128, W], f32)
                if BOUNCE_BC:
                    nc.sync.dma_start(wvals[:],
                                      bass.AP(brf_d, 0, [[0, 128], [1, W]]))
                else:
                    nc.gpsimd.partition_broadcast(wvals[:], wexp[:],
                                                  channels=128)
                wb = sp.tile([128, W], bf16)
                nc.vector.tensor_copy(wb[:], wvals[:])

                # local_scatter builds G16 [128, 16*128] in SBUF (zeroes dst,
                # then dst[p, idx[p,j]] = wb[p,j]); 2 chunks of 1024 columns
                # to fit the GPSIMD local-RAM limit; out-of-chunk indices go
                # negative (ignored).
                G16 = sp.tile([128, 2048], bf16)
                i16 = mybir.dt.int16
                tall = sp.tile([128, 2 * W], i32)
                nc.vector.tensor_tensor(
                    ap3(tall, [[W, 2], [1, W]]),
                    ap3(offc, [[0, 2], [1, W]]),
                    ap3(coff, [[W, 2], [1, W]]), ALU.subtract)
                gall = sp.tile([128, 2 * W], i32)
                nc.vector.tensor_scalar(gall[:], tall[:], 1024, None,
                                        ALU.is_ge)
                nc.vector.tensor_scalar(gall[:], gall[:], 8192, None,
                                        ALU.mult)
                nc.vector.tensor_tensor(tall[:], tall[:], gall[:],
                                        ALU.subtract)
                idx16 = sp.tile([128, 2 * W], i16)
                nc.vector.tensor_copy(idx16[:], tall[:])
                for c in range(2):
                    nc.gpsimd.local_scatter(
                        G16[:, 1024 * c:1024 * (c + 1)], wb[:],
                        idx16[:, W * c:W * (c + 1)],
                        channels=128, num_elems=1024, num_idxs=W)
                # source-block indices -> PE sequencer registers
                svals = [
                    nc.values_load(
                        blk[:, j:j + 1], engines=[mybir.EngineType.PE],
                        min_val=0, max_val=31,
                        skip_runtime_bounds_check=True)
                    for j in range(TOPK)
                ]

                if True:
                    # --------- Phase 9: compact dynamic-block output ---------
                    # 16 matmuls per tile: candidate j's slot pair (2j, 2j+1)
                    # contracts against VT blocks (b_j+u) and (b_j+u+1), whose
                    # offsets come from PE registers (no mod: VT duplicated).
                    with tc.tile_pool(name="ost", bufs=4) as ostp, \
                         tc.tile_pool(name="ops", bufs=2, space="PSUM") as opsp:
                        for u in range(NT):
                            ops = opsp.tile([128, C], f32, tag="o")
                            for j in range(TOPK):
                                for h in range(2):
                                    sl = 2 * j + h
                                    rhs = VT[:, bass.ds(
                                        (svals[j] + (u + h)) * C, C)]
                                    nc.tensor.matmul(
                                        ops[:],
                                        G16[:, 128 * sl:128 * (sl + 1)],
                                        rhs,
                                        start=(sl == 0), stop=(sl == 15),
                                        skip_group_check=True)
                            og = ostp.tile([128, C], f32, tag="og")
                            nc.scalar.activation(og[:], ops[:], AF.Identity,
                                                 scale=wrecb[:, 0:1])
                            nc.sync.dma_start(o_d[128 * u:128 * (u + 1), :], og[:])

    nc.finalize()
    return nc


def _get_nc():
    if "nc" not in _CACHE:
        _CACHE["nc"] = _build()
    return _CACHE["nc"]


def kernel(queries, keys, values):
    from concourse import bass_utils

    nc = _get_nc()
    b, l, h, e = queries.shape
    assert (b, l, h, e) == (B, L, 8, 64)
    ident = np.eye(128, dtype=np.float32)
    in_maps = []
    for i in range(B):
        in_maps.append({
            "q": np.ascontiguousarray(queries[i].reshape(L, C), dtype=np.float32),
            "k": np.ascontiguousarray(keys[i].reshape(L, C), dtype=np.float32),
            "v": np.ascontiguousarray(values[i].reshape(L, C), dtype=np.float32),
            "ident": ident,
        })
    trace = os.environ.get("AC_TRACE", "0") == "1"
    res = bass_utils.run_bass_kernel_spmd(
        nc, in_maps, core_ids=list(range(B)), trace=trace)
    if res.exec_time_ns is not None:
        kernel.last_exec_time_ns = res.exec_time_ns
        print(f"[kernel] HW exec time: {res.exec_time_ns} ns", file=sys.stderr)
    out = np.stack([res.results[i]["o"].reshape(L, h, e) for i in range(B)])
    return out


kernel.last_exec_time_ns = None



# revision 35
# speedup vs baseline: 1.2669x; 1.0195x over previous
"""AutoCorrelation (Autoformer) Bass kernel for 8 trn2 NeuronCores.

Problem: B=8, L=4096, H=8, E=64, TOP_K=8.
Sharding: data-parallel over batch (core b handles batch b); the cross-batch
mean for top-k index selection is a [4096]-element AllReduce.

Per-core algorithm (batch slice q,k,v: [L=4096, C=512] fp32, C = H*E):
  1. q,k tile loads interleaved; TensorE transposes -> qT,kT [C, L] bf16
     (PSUM->SBUF copies split across Scalar and Vector engines). The first
     correlation group is woven into the load loop so the PE ramps while DMA
     streams; v is prefetched and cast to bf16 on DVE during the correlation.
  2. Block-Toeplitz correlation on PE (bf16): for block offset m in [0,32):
       D_m[i,j] = sum_{u,c} qT[c,128u+i] * kT[c, 128((u+m)%32)+j]
     accumulated in PSUM tiles [128, 512] (4 block offsets per tile).
  3. mean_value[128m+d] = sum_i [D_m|D_{m+1}][i, i+d]: diagonal sums extracted
     with a DRAM "skewed-pitch" bounce (write pitch 4224, read pitch 4225),
     reads and ones-matmul means software-pipelined into later corr groups.
  4. AllReduce mean_value over the 8 cores in 3 slices; the first two (and
     their slice-top-8) hide under the correlation. Top-8 merge by threshold:
     24 slice candidates, the 16 losers keep weight 0. Per-batch weights
     gathered by a wide iota-compare masked reduce, summed across partitions
     with a ones matmul; exp() folded into the PSUM drain, softmax
     normalization folded into the output copies as a 1/sum scale.
  5. Output = sum_j w_j * roll(v, -d_j) as a 33-block circulant matmul in
     bf16. G [128, 4224] built in SBUF by gpsimd local_scatter of the
     bf16 exp-weights at diagonal offsets computed on DVE.
"""

import os
import sys
import numpy as np

sys.path.insert(0, "/opt/trn_rl_repo")

L = 4096
C = 512  # H*E
B = 8
NT = 32  # L/128 tiles
TOPK = 8
SCALE = 1.0 / 512.0  # mean over H*E
GW = 4224  # 33*128 circulant block columns

_CACHE = {}


def _build():
    import concourse.bass as bass
    import concourse.tile as tile
    from concourse import bacc, mybir

    f32 = mybir.dt.float32
    bf16 = mybir.dt.bfloat16
    i32 = mybir.dt.int32
    u32 = mybir.dt.uint32
    AF = mybir.ActivationFunctionType
    ALU = mybir.AluOpType

    nc = bacc.Bacc(
        "TRN2", target_bir_lowering=False, debug=False, num_devices=B,
    )

    q_d = nc.dram_tensor("q", [L, C], f32, kind="ExternalInput")
    k_d = nc.dram_tensor("k", [L, C], f32, kind="ExternalInput")
    v_d = nc.dram_tensor("v", [L, C], f32, kind="ExternalInput")
    ident_d = nc.dram_tensor("ident", [128, 128], f32, kind="ExternalInput")
    o_d = nc.dram_tensor("o", [L, C], f32, kind="ExternalOutput")

    # DRAM scratch
    ed_d = nc.dram_tensor("ed", [128 * GW + 128], f32, kind="Internal")
    bri_d = nc.dram_tensor("bri", [32], i32, kind="Internal")
    brf_d = nc.dram_tensor("brf", [32], f32, kind="Internal")
    ccin_d = nc.dram_tensor("ccin", [L], f32, kind="Internal")
    ccout_d = nc.dram_tensor("ccout", [L], f32, kind="Internal",
                             addr_space="Shared")
    cand_d = nc.dram_tensor("cand", [1024], f32, kind="Internal")
    bar_i = nc.dram_tensor("bar_i", [1], mybir.dt.uint8, kind="Internal")
    bar_o = nc.dram_tensor("bar_o", [B], mybir.dt.uint8, kind="Internal")

    with tile.TileContext(nc) as tc:
        with tc.tile_pool(name="const", bufs=1) as constp, \
             tc.tile_pool(name="mvp", bufs=1) as mvp, \
             tc.tile_pool(name="vtp", bufs=1) as vtp:
            ident = constp.tile([128, 128], f32)
            nc.sync.dma_start(ident[:], ident_d[:, :])
            identb = constp.tile([128, 128], bf16)
            nc.vector.tensor_copy(identb[:], ident[:])
            ones = constp.tile([128, 1], f32)
            nc.vector.memset(ones[:], SCALE)
            onesw = constp.tile([128, 1], f32)
            nc.vector.memset(onesw[:], 1.0)
            ones128 = constp.tile([1, 128], f32)
            nc.vector.memset(ones128[:], 1.0)
            # gpsimd local_scatter pre-warm operands (dummy run mid-corr
            # keeps the scatter program resident so the real calls skip the
            # multi-us dispatch latency)
            pwsrc = constp.tile([128, 2], bf16)
            nc.vector.memset(pwsrc[:], 0.0)
            pwidx = constp.tile([128, 2], mybir.dt.int16)
            nc.vector.memset(pwidx[:], 0)
            pwdst = constp.tile([128, 128], bf16)
            mv = mvp.tile([1, L], f32)
            # v tiles, bf16, tile-major, duplicated (blocks 0..31, 0..31) so
            # the output phase can take dynamic 512-col slices without mod-32
            # wraparound handling.
            VT = vtp.tile([128, 2 * NT * C], bf16)

            W = 8  # top-k candidates carried through the mid-section

            # data-independent constants for the mid-section (built early,
            # off the critical path):
            # ioWf[p, 32j+f] = 32p + f  (weight-gather compare basis)
            ioW = constp.tile([128, 32 * W], i32)
            nc.gpsimd.iota(ioW[:], [[0, W], [1, 32]], base=0,
                           channel_multiplier=32)
            ioWf = constp.tile([128, 32 * W], f32)
            nc.vector.tensor_copy(ioWf[:], ioW[:])
            # tpW[p, j] = p  (partition index)
            tpW = constp.tile([128, W], i32)
            nc.gpsimd.iota(tpW[:], [[0, W]], base=0, channel_multiplier=1)
            # coff[p, W*c+j] = 1024c (local_scatter chunk offsets)
            coff = constp.tile([128, 2 * W], i32)
            nc.gpsimd.iota(coff[:], [[1024, 2], [0, W]], base=0,
                           channel_multiplier=0)
            # joff[p, j] = 256j (compact G16 slot-pair base columns)
            joff = constp.tile([128, W], i32)
            nc.gpsimd.iota(joff[:], [[256, W]], base=0,
                           channel_multiplier=0)
            # io32[p, c] = 32p + c (global delay index for the enc-topk)
            io32 = constp.tile([128, 32], i32)
            nc.gpsimd.iota(io32[:], [[1, 32]], base=0, channel_multiplier=32)
            # mid-section tiles that are produced during the corr phase
            cidx = constp.tile([1, W], i32)

            # warm-up AllGather: spins up the ncfw collective pipeline during
            # the load phase so the mid-section AllReduce slices skip the
            # ~12us cold-launch latency.
            nc.gpsimd.collective_compute(
                "AllGather", mybir.AluOpType.bypass,
                replica_groups=[list(range(B))],
                ins=[bar_i[:]], outs=[bar_o[:]])

            # -------- Phase 1+2: pipelined load/transpose/correlation -------
            # q,k tile loads interleaved; the first correlation group (mg=0)
            # is woven into the load loop so PE ramps while DMA streams; the
            # shear bounce + mean matmuls are interleaved into the mg loop.
            with tc.tile_pool(name="qkT", bufs=1) as qkTp, \
                 tc.tile_pool(name="ld", bufs=6) as ldp, \
                 tc.tile_pool(name="sh", bufs=4) as shp, \
                 tc.tile_pool(name="trps", bufs=4, space="PSUM") as trpsp, \
                 tc.tile_pool(name="corr", bufs=2, space="PSUM") as corrp, \
                 tc.tile_pool(name="mvps", bufs=2, space="PSUM") as mvpsp, \
                 tc.tile_pool(name="pp", bufs=2) as ppp:
                # q/k transposed bf16, group-major in one tile per tensor
                qTall = qkTp.tile([128, 4 * L], bf16, name="qTall")
                kTall = qkTp.tile([128, 4 * L], bf16, name="kTall")

                def corr_mms(P, mg, u, first):
                    s = 128 * ((u + mg) % NT)
                    last = (u == NT - 1)
                    for g in range(4):
                        lhsT = kTall[:, g * L + 128 * u:g * L + 128 * (u + 1)]
                        st = first and g == 0
                        lastg = last and g == 3
                        if s <= L - 512:
                            nc.tensor.matmul(
                                P[:, :], lhsT,
                                qTall[:, g * L + s:g * L + s + 512],
                                start=st, stop=lastg,
                                skip_group_check=True)
                        else:
                            n1 = L - s
                            nc.tensor.matmul(
                                P[:, 0:n1], lhsT,
                                qTall[:, g * L + s:g * L + L],
                                start=st, stop=False,
                                skip_group_check=True)
                            nc.tensor.matmul(
                                P[:, n1:512], lhsT,
                                qTall[:, g * L:g * L + 512 - n1],
                                start=st, stop=lastg,
                                skip_group_check=True)

                P0 = corrp.tile([128, 512], f32, tag="P", name="P0")
                P1 = corrp.tile([128, 512], f32, tag="P", name="P1")
                for u in range(NT):
                    for (src, dstT, ceng) in ((q_d, qTall, nc.scalar),
                                              (k_d, kTall, nc.vector)):
                        t_in = ldp.tile([128, C], f32, tag="ld")
                        nc.sync.dma_start(t_in[:], src[128 * u:128 * (u + 1), :])
                        # cast to bf16 on the opposite engine from the copies
                        # so the transposes run at 1 cycle/row instead of 2
                        t_b = ldp.tile([128, C], bf16, tag="ldb")
                        if ceng is nc.scalar:
                            nc.vector.tensor_copy(t_b[:], t_in[:])
                        else:
                            nc.scalar.activation(t_b[:], t_in[:], AF.Identity)
                        for g in range(4):
                            ps = trpsp.tile([128, 128], bf16, tag="tr")
                            nc.tensor.transpose(
                                ps[:], t_b[:, 128 * g:128 * (g + 1)], identb[:])
                            # PSUM->SBUF copy casts bf16 -> fp8e4
                            dsl = dstT[:, g * L + 128 * u:g * L + 128 * (u + 1)]
                            if ceng is nc.scalar:
                                nc.scalar.activation(dsl, ps[:], AF.Identity)
                            else:
                                nc.vector.tensor_copy(dsl, ps[:])
                    # weave corr groups mg=0 and mg=4 behind the loads: the
                    # window for (mg, u_w) needs q tiles u_w+mg/128..+3 and k
                    # tile u_w, all loaded by iteration u_w+mg/128+3.
                    if u >= 3 and u - 3 <= NT - 5:
                        corr_mms(P0, 0, u - 3, first=(u == 3))
                    if u >= 7 and u - 7 <= NT - 8:
                        corr_mms(P1, 4, u - 7, first=(u == 7))

                # v prefetch: DMA queued behind q/k, casts on DVE (written to
                # both duplicate halves); completes during the corr matmuls.
                for u in range(NT):
                    t_in = ldp.tile([128, C], f32, tag="vld")
                    nc.sync.dma_start(t_in[:], v_d[128 * u:128 * (u + 1), :])
                    nc.vector.tensor_copy(VT[:, C * u:C * (u + 1)], t_in[:])
                    nc.vector.tensor_copy(
                        VT[:, C * (NT + u):C * (NT + u + 1)], t_in[:])

                def drain_chunk(P, ch):
                    PPt = ppp.tile([128, 512], f32, tag="pp")
                    nc.scalar.activation(PPt[:], P[:], AF.Identity)
                    # shear-write this chunk (pitch GW=4224)
                    nc.sync.dma_start(
                        bass.AP(ed_d, 512 * ch, [[GW, 128], [1, 512]]),
                        PPt[:])
                    if ch == 0:
                        # wrap block: ed cols [4096:4224] = chunk0 cols [0:128]
                        nc.sync.dma_start(
                            bass.AP(ed_d, L, [[GW, 128], [1, 128]]),
                            PPt[:, 0:128])

                SHs = {}

                def shear_read(ch):
                    # skewed read: pitch GW+1 extracts diagonals; chunk ch
                    # touches write-chunks ch and ch+1 (skew <= 127), so this
                    # must be issued after drain_chunk(ch+1).
                    SH = shp.tile([128, 512], f32, tag="sh")
                    nc.sync.dma_start(
                        SH[:],
                        bass.AP(ed_d, 512 * ch, [[GW + 1, 128], [1, 512]]))
                    SHs[ch] = SH

                def mean_mm(ch):
                    mps = mvpsp.tile([1, 512], f32, tag="mv")
                    nc.tensor.matmul(
                        mps[:], ones[:], SHs[ch][:], start=True, stop=True)
                    nc.scalar.activation(
                        mv[:, 512 * ch:512 * (ch + 1)], mps[:], AF.Identity)

                # finish mg=0 (wrap rows), then the remaining 7 groups.
                # Shear chunk ch-1 is read back right after drain(ch) lands;
                # its mean-matmul runs mid-way through the NEXT group so it
                # never heads the PE queue while the DMA is in flight.
                for u in range(NT - 4, NT):
                    corr_mms(P0, 0, u, first=False)
                drain_chunk(P0, 0)
                for u in range(NT - 7, NT):
                    corr_mms(P1, 4, u, first=False)
                drain_chunk(P1, 1)
                shear_read(0)
                for mg in range(8, NT, 4):
                    P = corrp.tile([128, 512], f32, tag="P")
                    ch = mg // 4
                    for u in range(NT):
                        corr_mms(P, mg, u, first=(u == 0))
                        if u == 16 and ch >= 2:
                            mean_mm(ch - 2)
                        if u == 24 and ch == 5:
                            # slice A (delays 0..2047, mean chunks 0..3) of
                            # the AllReduce launches mid-corr and hides fully
                            nc.sync.dma_start(ccin_d[0:2048], mv[:, 0:2048])
                            nc.gpsimd.collective_compute(
                                "AllReduce", mybir.AluOpType.add,
                                replica_groups=[list(range(B))],
                                ins=[ccin_d[0:2048]], outs=[ccout_d[0:2048]],
                            )
                        if u == 28 and ch == 5:
                            # pre-warm the gpsimd scatter program (hidden)
                            nc.gpsimd.local_scatter(
                                pwdst[:], pwsrc[:], pwidx[:],
                                channels=128, num_elems=128, num_idxs=2)
                    drain_chunk(P, ch)
                    shear_read(ch - 1)
                shear_read(7)
                mean_mm(6)
                mean_mm(7)

            # ------- Phase 5: AllReduce slice B (delays 2048..4095) -------
            nc.sync.dma_start(ccin_d[2048:L], mv[:, 2048:L])
            nc.gpsimd.collective_compute(
                "AllReduce", mybir.AluOpType.add,
                replica_groups=[list(range(B))],
                ins=[ccin_d[2048:L]], outs=[ccout_d[2048:L]],
            )
            with tc.tile_pool(name="small", bufs=1) as sp:
                # per-batch mean reshaped [128, 32] for the weight gather
                # (local values only)
                mv2 = sp.tile([128, 32], f32)
                nc.sync.dma_start(mv2[:], bass.AP(ccin_d, 0, [[32, 128], [1, 32]]))

                # ---------------- Phase 6: top-8 ----------------
                # delay d lives at [partition d>>5, col d&31]; slice A covers
                # partitions 0..63, slice B 64..127. Select top-8 of the
                # summed mean via an encoded (value<<12 | delay) per-partition
                # max8 + flat 1024 rescan.
                grand2 = sp.tile([128, 32], f32)
                nc.sync.dma_start(grand2[0:64, :],
                                  bass.AP(ccout_d, 0, [[32, 64], [1, 32]]))
                nc.sync.dma_start(grand2[64:128, :],
                                  bass.AP(ccout_d, 2048, [[32, 64], [1, 32]]))
                encf = sp.tile([128, 32], f32)
                nc.vector.tensor_scalar(encf[:], grand2[:], 32.0, 2048.0,
                                        ALU.mult, ALU.add)
                enci = sp.tile([128, 32], i32)
                nc.vector.tensor_copy(enci[:], encf[:])  # rounds
                nc.vector.tensor_scalar(enci[:], enci[:], 4096, None,
                                        ALU.mult)
                nc.vector.tensor_tensor(enci[:], enci[:], io32[:], ALU.add)
                nc.vector.tensor_copy(encf[:], enci[:])  # exact (< 2^24)
                c8 = sp.tile([128, 8], f32)
                c8i = sp.tile([128, 8], u32)
                nc.vector.max_with_indices(c8[:], c8i[:], encf[:])
                nc.sync.dma_start(cand_d[:], c8[:])
                cflat = sp.tile([1, 1024], f32)
                nc.sync.dma_start(cflat[:], bass.AP(cand_d, 0, [[1024, 1], [1, 1024]]))
                t8 = sp.tile([1, 8], f32)
                t8i = sp.tile([1, 8], u32)
                nc.vector.max_with_indices(t8[:], t8i[:], cflat[:])
                enc8 = sp.tile([1, 8], i32)
                nc.vector.tensor_copy(enc8[:], t8[:])
                nc.vector.tensor_scalar(cidx[:], enc8[:], 4095, None,
                                        ALU.bitwise_and)
                # per-candidate source block index for the dynamic VT slices;
                # loaded into PE registers early (one packed reg_load) so the
                # PE sequencer resolves them while DVE builds the weights.
                blk = sp.tile([1, W], i32)
                nc.vector.tensor_scalar(blk[:], cidx[:], 7, None,
                                        ALU.arith_shift_right)
                _, svals = nc.values_load_multi_w_load_instructions(
                    blk[:, 0:W], engines=[mybir.EngineType.PE],
                    min_val=0, max_val=31, skip_runtime_bounds_check=True)

                # ------------- Phase 7: weights + softmax -------------
                # partition broadcast of the raw enc floats via a K=1 PE
                # outer-product (ones128^T @ t8): ~100ns, no DRAM bounce.
                with tc.tile_pool(name="mid", bufs=2, space="PSUM") as midp:
                    t8ps = midp.tile([128, 8], f32, tag="bc")
                    nc.tensor.matmul(t8ps[:], ones128[:], t8[:],
                                     start=True, stop=True)
                    t8b = sp.tile([128, 8], f32)
                    nc.scalar.activation(t8b[:], t8ps[:], AF.Identity)
                encb = sp.tile([128, W], i32)
                nc.vector.tensor_copy(encb[:], t8b[:])
                idxb = sp.tile([128, W], i32)
                nc.vector.tensor_scalar(idxb[:], encb[:], 4095, None,
                                        ALU.bitwise_and)
                idxf = sp.tile([128, W], f32)
                nc.vector.tensor_copy(idxf[:], idxb[:])

                # wpW[p, j] = sum_f [32p+f == d_j] * mv2[p, f] in one wide
                # masked-reduce over [128, W, 32] broadcast access patterns.
                def ap3(t, d):
                    return bass.AP(t.tensor, 0, [[t.shape[1], 128]] + d)
                cmpm = sp.tile([128, 32 * W], f32)
                nc.vector.tensor_tensor(
                    ap3(cmpm, [[32, W], [1, 32]]),
                    ap3(ioWf, [[32, W], [1, 32]]),
                    ap3(idxf, [[1, W], [0, 32]]), ALU.is_equal)
                prods = sp.tile([128, 32 * W], f32)
                nc.vector.tensor_tensor(
                    ap3(prods, [[32, W], [1, 32]]),
                    ap3(cmpm, [[32, W], [1, 32]]),
                    ap3(mv2, [[0, W], [1, 32]]), ALU.mult)
                wpW = sp.tile([128, W], f32)
                nc.vector.tensor_reduce(
                    wpW[:], ap3(prods, [[32, W], [1, 32]]),
                    mybir.AxisListType.X, ALU.add)
                # cross-partition sum via ones matmul; Exp folded into the
                # PSUM drain. Softmax normalization is folded into the output
                # copies (scale by 1/sum), so G holds unnormalized exp(w).
                with tc.tile_pool(name="wps", bufs=2, space="PSUM") as wpsp:
                    wps = wpsp.tile([1, W], f32, tag="w")
                    nc.tensor.matmul(wps[:], onesw[:], wpW[:],
                                     start=True, stop=True)
                    wexp = sp.tile([1, W], f32)
                    nc.scalar.activation(wexp[:], wps[:], AF.Exp)
                    wsum = sp.tile([1, 1], f32)
                    nc.vector.tensor_reduce(wsum[:], wexp[:],
                                            mybir.AxisListType.X, ALU.add)
                    wrec = sp.tile([1, 1], f32)
                    nc.vector.reciprocal(wrec[:], wsum[:])
                    # broadcast exp(w) (8) and 1/sum (1) to all partitions in
                    # one K=1 PE outer-product
                    wcat = sp.tile([1, 9], f32)
                    nc.vector.tensor_copy(wcat[:, 0:8], wexp[:])
                    nc.vector.tensor_copy(wcat[:, 8:9], wrec[:])
                    wcps = wpsp.tile([128, 9], f32, tag="w")
                    nc.tensor.matmul(wcps[:], ones128[:], wcat[:],
                                     start=True, stop=True)
                    wcb = sp.tile([128, 9], f32)
                    nc.scalar.activation(wcb[:], wcps[:], AF.Identity)
                wrecb = wcb[:, 8:9]
                wb = sp.tile([128, W], bf16)
                nc.vector.tensor_copy(wb[:], wcb[:, 0:8])

                # -------- Phase 8: G16 diag offsets + local_scatter --------
                # Compact slot layout: candidate j owns slots (2j, 2j+1) =
                # source blocks (b_j, b_j+1); within the 256-col slot pair the
                # scatter position is ((p - r_j) & 255), identical banded
                # semantics to the full circulant but only 16 blocks.
                # offc[p,j] = 256*j + ((p - (d & 127)) & 255)
                rj = sp.tile([128, W], i32)
                nc.vector.tensor_scalar(rj[:], idxb[:], 127, None,
                                        ALU.bitwise_and)
                tmr = sp.tile([128, W], i32)
                nc.vector.tensor_tensor(tmr[:], tpW[:], rj[:], ALU.subtract)
                nc.vector.tensor_scalar(tmr[:], tmr[:], 255, None,
                                        ALU.bitwise_and)
                offc = sp.tile([128, W], i32)
                nc.vector.tensor_tensor(offc[:], joff[:], tmr[:], ALU.add)

                # local_scatter builds G16 [128, 16*128] in SBUF (zeroes dst,
                # then dst[p, idx[p,j]] = wb[p,j]); 2 chunks of 1024 columns
                # to fit the GPSIMD local-RAM limit; out-of-chunk indices go
                # negative (ignored).
                G16 = sp.tile([128, 2048], bf16)
                i16 = mybir.dt.int16
                tall = sp.tile([128, 2 * W], i32)
                nc.vector.tensor_tensor(
                    ap3(tall, [[W, 2], [1, W]]),
                    ap3(offc, [[0, 2], [1, W]]),
                    ap3(coff, [[W, 2], [1, W]]), ALU.subtract)
                gall = sp.tile([128, 2 * W], i32)
                nc.vector.tensor_scalar(gall[:], tall[:], 1024, None,
                                        ALU.is_ge)
                nc.vector.tensor_scalar(gall[:], gall[:], 8192, None,
                                        ALU.mult)
                nc.vector.tensor_tensor(tall[:], tall[:], gall[:],
                                        ALU.subtract)
                idx16 = sp.tile([128, 2 * W], i16)
                nc.vector.tensor_copy(idx16[:], tall[:])
                for c in range(2):
                    nc.gpsimd.local_scatter(
                        G16[:, 1024 * c:1024 * (c + 1)], wb[:],
                        idx16[:, W * c:W * (c + 1)],
                        channels=128, num_elems=1024, num_idxs=W)

                if True:
                    # --------- Phase 9: compact dynamic-block output ---------
                    # 16 matmuls per tile: candidate j's slot pair (2j, 2j+1)
                    # contracts against VT blocks (b_j+u) and (b_j+u+1), whose
                    # offsets come from PE registers (no mod: VT duplicated).
                    with tc.tile_pool(name="ost", bufs=4) as ostp, \
                         tc.tile_pool(name="ops", bufs=2, space="PSUM") as opsp:
                        for u in range(NT):
                            ops = opsp.tile([128, C], f32, tag="o")
                            for j in range(TOPK):
                                for h in range(2):
                                    sl = 2 * j + h
                                    rhs = VT[:, bass.ds(
                                        (svals[j] + (u + h)) * C, C)]
                                    nc.tensor.matmul(
                                        ops[:],
                                        G16[:, 128 * sl:128 * (sl + 1)],
                                        rhs,
                                        start=(sl == 0), stop=(sl == 15),
                                        skip_group_check=True)
                            og = ostp.tile([128, C], f32, tag="og")
                            nc.scalar.activation(og[:], ops[:], AF.Identity,
                                                 scale=wrecb)
                            nc.sync.dma_start(o_d[128 * u:128 * (u + 1), :], og[:])

    nc.finalize()
    return nc


def _get_nc():
    if "nc" not in _CACHE:
        _CACHE["nc"] = _build()
    return _CACHE["nc"]


def kernel(queries, keys, values):
    from concourse import bass_utils

    nc = _get_nc()
    b, l, h, e = queries.shape
    assert (b, l, h, e) == (B, L, 8, 64)
    ident = np.eye(128, dtype=np.float32)
    in_maps = []
    for i in range(B):
        in_maps.append({
            "q": np.ascontiguousarray(queries[i].reshape(L, C), dtype=np.float32),
            "k": np.ascontiguousarray(keys[i].reshape(L, C), dtype=np.float32),
            "v": np.ascontiguousarray(values[i].reshape(L, C), dtype=np.float32),
            "ident": ident,
        })
    trace = os.environ.get("AC_TRACE", "0") == "1"
    res = bass_utils.run_bass_kernel_spmd(
        nc, in_maps, core_ids=list(range(B)), trace=trace)
    if res.exec_time_ns is not None:
        kernel.last_exec_time_ns = res.exec_time_ns
        print(f"[kernel] HW exec time: {res.exec_time_ns} ns", file=sys.stderr)
    out = np.stack([res.results[i]["o"].reshape(L, h, e) for i in range(B)])
    return out


kernel.last_exec_time_ns = None



# revision 39
# speedup vs baseline: 1.2803x; 1.0106x over previous
"""AutoCorrelation (Autoformer) Bass kernel for 8 trn2 NeuronCores.

Problem: B=8, L=4096, H=8, E=64, TOP_K=8.
Sharding: data-parallel over batch (core b handles batch b); the cross-batch
mean for top-k index selection is a [4096]-element AllReduce.

Per-core algorithm (batch slice q,k,v: [L=4096, C=512] fp32, C = H*E):
  1. q,k tile loads interleaved; TensorE transposes -> qT,kT [C, L] bf16
     (PSUM->SBUF copies split across Scalar and Vector engines). The first
     correlation group is woven into the load loop so the PE ramps while DMA
     streams; v is prefetched and cast to bf16 on DVE during the correlation.
  2. Block-Toeplitz correlation on PE (bf16): for block offset m in [0,32):
       D_m[i,j] = sum_{u,c} qT[c,128u+i] * kT[c, 128((u+m)%32)+j]
     accumulated in PSUM tiles [128, 512] (4 block offsets per tile).
  3. mean_value[128m+d] = sum_i [D_m|D_{m+1}][i, i+d]: diagonal sums extracted
     with a DRAM "skewed-pitch" bounce (write pitch 4224, read pitch 4225),
     reads and ones-matmul means software-pipelined into later corr groups.
  4. AllReduce mean_value over the 8 cores in 3 slices; the first two (and
     their slice-top-8) hide under the correlation. Top-8 merge by threshold:
     24 slice candidates, the 16 losers keep weight 0. Per-batch weights
     gathered by a wide iota-compare masked reduce, summed across partitions
     with a ones matmul; exp() folded into the PSUM drain, softmax
     normalization folded into the output copies as a 1/sum scale.
  5. Output = sum_j w_j * roll(v, -d_j) as a 33-block circulant matmul in
     bf16. G [128, 4224] built in SBUF by gpsimd local_scatter of the
     bf16 exp-weights at diagonal offsets computed on DVE.
"""

import os
import sys
import numpy as np

sys.path.insert(0, "/opt/trn_rl_repo")

L = 4096
C = 512  # H*E
B = 8
NT = 32  # L/128 tiles
TOPK = 8
SCALE = 1.0 / 512.0  # mean over H*E
GW = 4224  # 33*128 circulant block columns

_CACHE = {}


def _build():
    import concourse.bass as bass
    import concourse.tile as tile
    from concourse import bacc, mybir

    f32 = mybir.dt.float32
    bf16 = mybir.dt.bfloat16
    i32 = mybir.dt.int32
    u32 = mybir.dt.uint32
    AF = mybir.ActivationFunctionType
    ALU = mybir.AluOpType

    nc = bacc.Bacc(
        "TRN2", target_bir_lowering=False, debug=False, num_devices=B,
    )

    q_d = nc.dram_tensor("q", [L, C], f32, kind="ExternalInput")
    k_d = nc.dram_tensor("k", [L, C], f32, kind="ExternalInput")
    v_d = nc.dram_tensor("v", [L, C], f32, kind="ExternalInput")
    ident_d = nc.dram_tensor("ident", [128, 128], f32, kind="ExternalInput")
    o_d = nc.dram_tensor("o", [L, C], f32, kind="ExternalOutput")

    # DRAM scratch
    ed_d = nc.dram_tensor("ed", [128 * GW + 128], f32, kind="Internal")
    bri_d = nc.dram_tensor("bri", [32], i32, kind="Internal")
    brf_d = nc.dram_tensor("brf", [32], f32, kind="Internal")
    ccin_d = nc.dram_tensor("ccin", [L], f32, kind="Internal")
    ccout_d = nc.dram_tensor("ccout", [L], f32, kind="Internal",
                             addr_space="Shared")
    cand_d = nc.dram_tensor("cand", [1024], f32, kind="Internal")
    bar_i = nc.dram_tensor("bar_i", [1], mybir.dt.uint8, kind="Internal")
    bar_o = nc.dram_tensor("bar_o", [B], mybir.dt.uint8, kind="Internal")

    with tile.TileContext(nc) as tc:
        with tc.tile_pool(name="const", bufs=1) as constp, \
             tc.tile_pool(name="mvp", bufs=1) as mvp, \
             tc.tile_pool(name="vtp", bufs=1) as vtp:
            ident = constp.tile([128, 128], f32)
            nc.sync.dma_start(ident[:], ident_d[:, :])
            identb = constp.tile([128, 128], bf16)
            nc.vector.tensor_copy(identb[:], ident[:])
            ones = constp.tile([128, 1], f32)
            nc.vector.memset(ones[:], SCALE)
            onesw = constp.tile([128, 1], f32)
            nc.vector.memset(onesw[:], 1.0)
            ones128 = constp.tile([1, 128], f32)
            nc.vector.memset(ones128[:], 1.0)
            # gpsimd local_scatter pre-warm operands (dummy run mid-corr
            # keeps the scatter program resident so the real calls skip the
            # multi-us dispatch latency)
            pwsrc = constp.tile([128, 2], bf16)
            nc.vector.memset(pwsrc[:], 0.0)
            pwidx = constp.tile([128, 2], mybir.dt.int16)
            nc.vector.memset(pwidx[:], 0)
            pwdst = constp.tile([128, 128], bf16)
            mv = mvp.tile([1, L], f32)
            # v tiles, bf16, tile-major, duplicated (blocks 0..31, 0..31) so
            # the output phase can take dynamic 512-col slices without mod-32
            # wraparound handling.
            VT = vtp.tile([128, 2 * NT * C], bf16)

            W = 8  # top-k candidates carried through the mid-section

            # data-independent constants for the mid-section (built early,
            # off the critical path):
            # ioWf[p, 32j+f] = 32p + f  (weight-gather compare basis)
            ioW = constp.tile([128, 32 * W], i32)
            nc.gpsimd.iota(ioW[:], [[0, W], [1, 32]], base=0,
                           channel_multiplier=32)
            ioWf = constp.tile([128, 32 * W], f32)
            nc.vector.tensor_copy(ioWf[:], ioW[:])
            # tpW[p, j] = p  (partition index)
            tpW = constp.tile([128, W], i32)
            nc.gpsimd.iota(tpW[:], [[0, W]], base=0, channel_multiplier=1)
            # coff[p, W*c+j] = 1024c (local_scatter chunk offsets)
            coff = constp.tile([128, 2 * W], i32)
            nc.gpsimd.iota(coff[:], [[1024, 2], [0, W]], base=0,
                           channel_multiplier=0)
            # joff[p, j] = 256j (compact G16 slot-pair base columns)
            joff = constp.tile([128, W], i32)
            nc.gpsimd.iota(joff[:], [[256, W]], base=0,
                           channel_multiplier=0)
            # io32[p, c] = 32p + c (global delay index for the enc-topk)
            io32 = constp.tile([128, 32], i32)
            nc.gpsimd.iota(io32[:], [[1, 32]], base=0, channel_multiplier=32)
            # mid-section tiles that are produced during the corr phase
            cidx = constp.tile([1, W], i32)

            # warm-up AllGather: spins up the ncfw collective pipeline during
            # the load phase so the mid-section AllReduce slices skip the
            # ~12us cold-launch latency.
            nc.gpsimd.collective_compute(
                "AllGather", mybir.AluOpType.bypass,
                replica_groups=[list(range(B))],
                ins=[bar_i[:]], outs=[bar_o[:]])

            # -------- Phase 1+2: pipelined load/transpose/correlation -------
            # q,k tile loads interleaved; the first correlation group (mg=0)
            # is woven into the load loop so PE ramps while DMA streams; the
            # shear bounce + mean matmuls are interleaved into the mg loop.
            with tc.tile_pool(name="qkT", bufs=1) as qkTp, \
                 tc.tile_pool(name="ld", bufs=6) as ldp, \
                 tc.tile_pool(name="sh", bufs=4) as shp, \
                 tc.tile_pool(name="trps", bufs=4, space="PSUM") as trpsp, \
                 tc.tile_pool(name="corr", bufs=2, space="PSUM") as corrp, \
                 tc.tile_pool(name="mvps", bufs=2, space="PSUM") as mvpsp, \
                 tc.tile_pool(name="pp", bufs=2) as ppp:
                # q/k transposed bf16, group-major in one tile per tensor
                qTall = qkTp.tile([128, 4 * L], bf16, name="qTall")
                kTall = qkTp.tile([128, 4 * L], bf16, name="kTall")

                def corr_mms(P, mg, u, first):
                    s = 128 * ((u + mg) % NT)
                    last = (u == NT - 1)
                    for g in range(4):
                        lhsT = kTall[:, g * L + 128 * u:g * L + 128 * (u + 1)]
                        st = first and g == 0
                        lastg = last and g == 3
                        if s <= L - 512:
                            nc.tensor.matmul(
                                P[:, :], lhsT,
                                qTall[:, g * L + s:g * L + s + 512],
                                start=st, stop=lastg,
                                skip_group_check=True)
                        else:
                            n1 = L - s
                            nc.tensor.matmul(
                                P[:, 0:n1], lhsT,
                                qTall[:, g * L + s:g * L + L],
                                start=st, stop=False,
                                skip_group_check=True)
                            nc.tensor.matmul(
                                P[:, n1:512], lhsT,
                                qTall[:, g * L:g * L + 512 - n1],
                                start=st, stop=lastg,
                                skip_group_check=True)

                P0 = corrp.tile([128, 512], f32, tag="P", name="P0")
                P1 = corrp.tile([128, 512], f32, tag="P", name="P1")
                for u in range(NT):
                    for (src, dstT, ceng) in ((q_d, qTall, nc.scalar),
                                              (k_d, kTall, nc.vector)):
                        t_in = ldp.tile([128, C], f32, tag="ld")
                        nc.sync.dma_start(t_in[:], src[128 * u:128 * (u + 1), :])
                        # cast to bf16 on the opposite engine from the copies
                        # so the transposes run at 1 cycle/row instead of 2
                        t_b = ldp.tile([128, C], bf16, tag="ldb")
                        if ceng is nc.scalar:
                            nc.vector.tensor_copy(t_b[:], t_in[:])
                        else:
                            nc.scalar.activation(t_b[:], t_in[:], AF.Identity)
                        for g in range(4):
                            ps = trpsp.tile([128, 128], bf16, tag="tr")
                            nc.tensor.transpose(
                                ps[:], t_b[:, 128 * g:128 * (g + 1)], identb[:])
                            # PSUM->SBUF copy casts bf16 -> fp8e4
                            dsl = dstT[:, g * L + 128 * u:g * L + 128 * (u + 1)]
                            if ceng is nc.scalar:
                                nc.scalar.activation(dsl, ps[:], AF.Identity)
                            else:
                                nc.vector.tensor_copy(dsl, ps[:])
                    # weave corr groups mg=0 and mg=4 behind the loads: the
                    # window for (mg, u_w) needs q tiles u_w+mg/128..+3 and k
                    # tile u_w, all loaded by iteration u_w+mg/128+3; one
                    # extra iteration of slack keeps the PE queue off the
                    # just-written transpose outputs.
                    if u >= 4 and u - 4 <= NT - 6:
                        corr_mms(P0, 0, u - 4, first=(u == 4))
                    if u >= 8 and u - 8 <= NT - 9:
                        corr_mms(P1, 4, u - 8, first=(u == 8))

                # v prefetch: DMA queued behind q/k, casts on DVE (written to
                # both duplicate halves); completes during the corr matmuls.
                for u in range(NT):
                    t_in = ldp.tile([128, C], f32, tag="vld")
                    nc.sync.dma_start(t_in[:], v_d[128 * u:128 * (u + 1), :])
                    nc.vector.tensor_copy(VT[:, C * u:C * (u + 1)], t_in[:])
                    nc.vector.tensor_copy(
                        VT[:, C * (NT + u):C * (NT + u + 1)], t_in[:])

                def drain_chunk(P, ch):
                    PPt = ppp.tile([128, 512], f32, tag="pp")
                    nc.scalar.activation(PPt[:], P[:], AF.Identity)
                    # shear-write this chunk (pitch GW=4224)
                    nc.sync.dma_start(
                        bass.AP(ed_d, 512 * ch, [[GW, 128], [1, 512]]),
                        PPt[:])
                    if ch == 0:
                        # wrap block: ed cols [4096:4224] = chunk0 cols [0:128]
                        nc.sync.dma_start(
                            bass.AP(ed_d, L, [[GW, 128], [1, 128]]),
                            PPt[:, 0:128])

                SHs = {}

                def shear_read(ch):
                    # skewed read: pitch GW+1 extracts diagonals; chunk ch
                    # touches write-chunks ch and ch+1 (skew <= 127), so this
                    # must be issued after drain_chunk(ch+1).
                    SH = shp.tile([128, 512], f32, tag="sh")
                    nc.sync.dma_start(
                        SH[:],
                        bass.AP(ed_d, 512 * ch, [[GW + 1, 128], [1, 512]]))
                    SHs[ch] = SH

                def mean_mm(ch):
                    mps = mvpsp.tile([1, 512], f32, tag="mv")
                    nc.tensor.matmul(
                        mps[:], ones[:], SHs[ch][:], start=True, stop=True)
                    nc.scalar.activation(
                        mv[:, 512 * ch:512 * (ch + 1)], mps[:], AF.Identity)

                # finish mg=0 (wrap rows), then the remaining 7 groups.
                # Shear chunk ch-1 is read back right after drain(ch) lands;
                # its mean-matmul runs mid-way through the NEXT group so it
                # never heads the PE queue while the DMA is in flight.
                for u in range(NT - 5, NT):
                    corr_mms(P0, 0, u, first=False)
                drain_chunk(P0, 0)
                for u in range(NT - 8, NT):
                    corr_mms(P1, 4, u, first=False)
                drain_chunk(P1, 1)
                shear_read(0)
                for mg in range(8, NT, 4):
                    P = corrp.tile([128, 512], f32, tag="P")
                    ch = mg // 4
                    for u in range(NT):
                        corr_mms(P, mg, u, first=(u == 0))
                        if u == 16 and ch >= 2:
                            mean_mm(ch - 2)
                        if u == 24 and ch == 5:
                            # slice A (delays 0..2047, mean chunks 0..3) of
                            # the AllReduce launches mid-corr and hides fully
                            nc.sync.dma_start(ccin_d[0:2048], mv[:, 0:2048])
                            nc.gpsimd.collective_compute(
                                "AllReduce", mybir.AluOpType.add,
                                replica_groups=[list(range(B))],
                                ins=[ccin_d[0:2048]], outs=[ccout_d[0:2048]],
                            )
                        if u == 28 and ch == 5:
                            # pre-warm the gpsimd scatter program (hidden)
                            nc.gpsimd.local_scatter(
                                pwdst[:], pwsrc[:], pwidx[:],
                                channels=128, num_elems=128, num_idxs=2)
                    drain_chunk(P, ch)
                    shear_read(ch - 1)
                shear_read(7)
                mean_mm(6)
                mean_mm(7)

            # ------- Phase 5: AllReduce slice B (delays 2048..4095) -------
            nc.sync.dma_start(ccin_d[2048:L], mv[:, 2048:L])
            nc.gpsimd.collective_compute(
                "AllReduce", mybir.AluOpType.add,
                replica_groups=[list(range(B))],
                ins=[ccin_d[2048:L]], outs=[ccout_d[2048:L]],
            )
            with tc.tile_pool(name="small", bufs=1) as sp:
                # per-batch mean reshaped [128, 32] for the weight gather
                # (local values only)
                mv2 = sp.tile([128, 32], f32)
                nc.sync.dma_start(mv2[:], bass.AP(ccin_d, 0, [[32, 128], [1, 32]]))

                # ---------------- Phase 6: top-8 ----------------
                # delay d lives at [partition d>>5, col d&31]; slice A covers
                # partitions 0..63, slice B 64..127. Select top-8 of the
                # summed mean via an encoded (value<<12 | delay) per-partition
                # max8 + flat 1024 rescan.
                grand2 = sp.tile([128, 32], f32)
                nc.sync.dma_start(grand2[0:64, :],
                                  bass.AP(ccout_d, 0, [[32, 64], [1, 32]]))
                nc.sync.dma_start(grand2[64:128, :],
                                  bass.AP(ccout_d, 2048, [[32, 64], [1, 32]]))
                encf = sp.tile([128, 32], f32)
                nc.vector.tensor_scalar(encf[:], grand2[:], 32.0, 2048.0,
                                        ALU.mult, ALU.add)
                enci = sp.tile([128, 32], i32)
                nc.vector.tensor_copy(enci[:], encf[:])  # rounds
                nc.vector.tensor_scalar(enci[:], enci[:], 4096, None,
                                        ALU.mult)
                nc.vector.tensor_tensor(enci[:], enci[:], io32[:], ALU.add)
                nc.vector.tensor_copy(encf[:], enci[:])  # exact (< 2^24)
                c8 = sp.tile([128, 8], f32)
                c8i = sp.tile([128, 8], u32)
                nc.vector.max_with_indices(c8[:], c8i[:], encf[:])
                nc.sync.dma_start(cand_d[:], c8[:])
                cflat = sp.tile([1, 1024], f32)
                nc.sync.dma_start(cflat[:], bass.AP(cand_d, 0, [[1024, 1], [1, 1024]]))
                t8 = sp.tile([1, 8], f32)
                t8i = sp.tile([1, 8], u32)
                nc.vector.max_with_indices(t8[:], t8i[:], cflat[:])
                enc8 = sp.tile([1, 8], i32)
                nc.vector.tensor_copy(enc8[:], t8[:])
                nc.vector.tensor_scalar(cidx[:], enc8[:], 4095, None,
                                        ALU.bitwise_and)
                # per-candidate source block index for the dynamic VT slices;
                # loaded into PE registers early (one packed reg_load) so the
                # PE sequencer resolves them while DVE builds the weights.
                blk = sp.tile([1, W], i32)
                nc.vector.tensor_scalar(blk[:], cidx[:], 7, None,
                                        ALU.arith_shift_right)
                _, svals = nc.values_load_multi_w_load_instructions(
                    blk[:, 0:W], engines=[mybir.EngineType.PE],
                    min_val=0, max_val=31, skip_runtime_bounds_check=True)

                # ------------- Phase 7: weights + softmax -------------
                # partition broadcast of the raw enc floats via a K=1 PE
                # outer-product (ones128^T @ t8): ~100ns, no DRAM bounce.
                with tc.tile_pool(name="mid", bufs=2, space="PSUM") as midp:
                    t8ps = midp.tile([128, 8], f32, tag="bc")
                    nc.tensor.matmul(t8ps[:], ones128[:], t8[:],
                                     start=True, stop=True)
                    t8b = sp.tile([128, 8], f32)
                    nc.scalar.activation(t8b[:], t8ps[:], AF.Identity)
                encb = sp.tile([128, W], i32)
                nc.vector.tensor_copy(encb[:], t8b[:])
                idxb = sp.tile([128, W], i32)
                nc.vector.tensor_scalar(idxb[:], encb[:], 4095, None,
                                        ALU.bitwise_and)
                idxf = sp.tile([128, W], f32)
                nc.vector.tensor_copy(idxf[:], idxb[:])

                # wpW[p, j] = sum_f [32p+f == d_j] * mv2[p, f] in one wide
                # masked-reduce over [128, W, 32] broadcast access patterns.
                def ap3(t, d):
                    return bass.AP(t.tensor, 0, [[t.shape[1], 128]] + d)
                cmpm = sp.tile([128, 32 * W], f32)
                nc.vector.tensor_tensor(
                    ap3(cmpm, [[32, W], [1, 32]]),
                    ap3(ioWf, [[32, W], [1, 32]]),
                    ap3(idxf, [[1, W], [0, 32]]), ALU.is_equal)
                prods = sp.tile([128, 32 * W], f32)
                nc.vector.tensor_tensor(
                    ap3(prods, [[32, W], [1, 32]]),
                    ap3(cmpm, [[32, W], [1, 32]]),
                    ap3(mv2, [[0, W], [1, 32]]), ALU.mult)
                wpW = sp.tile([128, W], f32)
                nc.vector.tensor_reduce(
                    wpW[:], ap3(prods, [[32, W], [1, 32]]),
                    mybir.AxisListType.X, ALU.add)
                # cross-partition sum via ones matmul; Exp folded into the
                # PSUM drain. Softmax normalization is folded into the output
                # copies (scale by 1/sum), so G holds unnormalized exp(w).
                with tc.tile_pool(name="wps", bufs=2, space="PSUM") as wpsp:
                    wps = wpsp.tile([1, W], f32, tag="w")
                    nc.tensor.matmul(wps[:], onesw[:], wpW[:],
                                     start=True, stop=True)
                    wexp = sp.tile([1, W], f32)
                    nc.scalar.activation(wexp[:], wps[:], AF.Exp)
                    wsum = sp.tile([1, 1], f32)
                    nc.vector.tensor_reduce(wsum[:], wexp[:],
                                            mybir.AxisListType.X, ALU.add)
                    wrec = sp.tile([1, 1], f32)
                    nc.vector.reciprocal(wrec[:], wsum[:])
                    # broadcast exp(w) (8) and 1/sum (1) to all partitions in
                    # one K=1 PE outer-product
                    wcat = sp.tile([1, 9], f32)
                    nc.vector.tensor_copy(wcat[:, 0:8], wexp[:])
                    nc.vector.tensor_copy(wcat[:, 8:9], wrec[:])
                    wcps = wpsp.tile([128, 9], f32, tag="w")
                    nc.tensor.matmul(wcps[:], ones128[:], wcat[:],
                                     start=True, stop=True)
                    wcb = sp.tile([128, 9], f32)
                    nc.scalar.activation(wcb[:], wcps[:], AF.Identity)
                wrecb = wcb[:, 8:9]
                wb = sp.tile([128, W], bf16)
                nc.vector.tensor_copy(wb[:], wcb[:, 0:8])

                # -------- Phase 8: G16 diag offsets + local_scatter --------
                # Compact slot layout: candidate j owns slots (2j, 2j+1) =
                # source blocks (b_j, b_j+1); within the 256-col slot pair the
                # scatter position is ((p - r_j) & 255), identical banded
                # semantics to the full circulant but only 16 blocks.
                # offc[p,j] = 256*j + ((p - (d & 127)) & 255)
                rj = sp.tile([128, W], i32)
                nc.vector.tensor_scalar(rj[:], idxb[:], 127, None,
                                        ALU.bitwise_and)
                tmr = sp.tile([128, W], i32)
                nc.vector.tensor_tensor(tmr[:], tpW[:], rj[:], ALU.subtract)
                nc.vector.tensor_scalar(tmr[:], tmr[:], 255, None,
                                        ALU.bitwise_and)
                offc = sp.tile([128, W], i32)
                nc.vector.tensor_tensor(offc[:], joff[:], tmr[:], ALU.add)

                # local_scatter builds G16 [128, 16*128] in SBUF (zeroes dst,
                # then dst[p, idx[p,j]] = wb[p,j]); 2 chunks of 1024 columns
                # to fit the GPSIMD local-RAM limit; out-of-chunk indices go
                # negative (ignored).
                G16 = sp.tile([128, 2048], bf16)
                i16 = mybir.dt.int16
                tall = sp.tile([128, 2 * W], i32)
                nc.vector.tensor_tensor(
                    ap3(tall, [[W, 2], [1, W]]),
                    ap3(offc, [[0, 2], [1, W]]),
                    ap3(coff, [[W, 2], [1, W]]), ALU.subtract)
                gall = sp.tile([128, 2 * W], i32)
                nc.vector.tensor_scalar(gall[:], tall[:], 1024, None,
                                        ALU.is_ge)
                nc.vector.tensor_scalar(gall[:], gall[:], 8192, None,
                                        ALU.mult)
                nc.vector.tensor_tensor(tall[:], tall[:], gall[:],
                                        ALU.subtract)
                idx16 = sp.tile([128, 2 * W], i16)
                nc.vector.tensor_copy(idx16[:], tall[:])
                for c in range(2):
                    nc.gpsimd.local_scatter(
                        G16[:, 1024 * c:1024 * (c + 1)], wb[:],
                        idx16[:, W * c:W * (c + 1)],
                        channels=128, num_elems=1024, num_idxs=W)

                if True:
                    # --------- Phase 9: compact dynamic-block output ---------
                    # 16 matmuls per tile: candidate j's slot pair (2j, 2j+1)
                    # contracts against VT blocks (b_j+u) and (b_j+u+1), whose
                    # offsets come from PE registers (no mod: VT duplicated).
                    with tc.tile_pool(name="ost", bufs=4) as ostp, \
                         tc.tile_pool(name="ops", bufs=2, space="PSUM") as opsp:
                        for u in range(NT):
                            ops = opsp.tile([128, C], f32, tag="o")
                            for j in range(TOPK):
                                for h in range(2):
                                    sl = 2 * j + h
                                    rhs = VT[:, bass.ds(
                                        (svals[j] + (u + h)) * C, C)]
                                    nc.tensor.matmul(
                                        ops[:],
                                        G16[:, 128 * sl:128 * (sl + 1)],
                                        rhs,
                                        start=(sl == 0), stop=(sl == 15),
                                        skip_group_check=True)
                            og = ostp.tile([128, C], f32, tag="og")
                            nc.scalar.activation(og[:], ops[:], AF.Identity,
                                                 scale=wrecb)
                            nc.sync.dma_start(o_d[128 * u:128 * (u + 1), :], og[:])

    nc.finalize()
    return nc


def _get_nc():
    if "nc" not in _CACHE:
        _CACHE["nc"] = _build()
    return _CACHE["nc"]


def kernel(queries, keys, values):
    from concourse import bass_utils

    nc = _get_nc()
    b, l, h, e = queries.shape
    assert (b, l, h, e) == (B, L, 8, 64)
    ident = np.eye(128, dtype=np.float32)
    in_maps = []
    for i in range(B):
        in_maps.append({
            "q": np.ascontiguousarray(queries[i].reshape(L, C), dtype=np.float32),
            "k": np.ascontiguousarray(keys[i].reshape(L, C), dtype=np.float32),
            "v": np.ascontiguousarray(values[i].reshape(L, C), dtype=np.float32),
            "ident": ident,
        })
    trace = os.environ.get("AC_TRACE", "0") == "1"
    res = bass_utils.run_bass_kernel_spmd(
        nc, in_maps, core_ids=list(range(B)), trace=trace)
    if res.exec_time_ns is not None:
        kernel.last_exec_time_ns = res.exec_time_ns
        print(f"[kernel] HW exec time: {res.exec_time_ns} ns", file=sys.stderr)
    out = np.stack([res.results[i]["o"].reshape(L, h, e) for i in range(B)])
    return out


kernel.last_exec_time_ns = None

